# revision 1
# baseline (speedup 1.0000x reference)
"""Trainium2 Bass kernel for nn_BaseModel_74302934220896 (TuckER + possibility-codebook).

Contract: kernel(**inputs) takes FULL unsharded inputs (as in reference.setup_inputs())
and returns the full output tuple (tucker_logits [B,N] f32, possibility_score [B,N] f32).

Sharding (8 cores):
  - B (2048) -> 8 x 256 for relation / hr / codebook-gather paths
  - N (20000) -> 8 x 2500 (padded to 2560) for tail features and the [B,N] score matmuls
  - head MLP replicated over full B on every core so BN0 needs no collective
  - ONE AllGather carries the per-core [WmT(raw); interT] shards; BN1 statistics are
    computed locally from the gathered full-B WmT.
"""

import sys

sys.path.insert(0, "/opt/trn_rl_repo")

import numpy as np
import ml_dtypes

import concourse.bass as bass
import concourse.bacc as bacc
import concourse.mybir as mybir
import concourse.tile as tile
from concourse.bass import IndirectOffsetOnAxis
from concourse.bass_utils import run_bass_kernel_spmd
from concourse.masks import make_identity

F32 = mybir.dt.float32
F32R = mybir.dt.float32r
BF16 = mybir.dt.bfloat16
I32 = mybir.dt.int32
AF = mybir.ActivationFunctionType
ALU = mybir.AluOpType
AX = mybir.AxisListType

B, N, E, C, R2 = 2048, 20000, 512, 128, 474
NCORES = 8
BSH = B // NCORES            # 256 b rows per core (sharded paths)
NSH = N // NCORES            # 2500 tail rows per core
NPAD = 2560                  # padded to 5 groups of 512
NG = NPAD // 512             # 5 n-groups
NB_FULL = B // 128           # 16 b-tiles over full B
TEMP = 0.5
NEG = -1.0e30

# dtype knobs
PC_DT = BF16                 # codebook gather + inter product dtype
WM_DT = BF16                 # Wm (rsT @ core2) matmul dtype
OUT_BF16 = False             # score outputs written as bf16 (upcast on host)
DEBUG = False                # add intermediate-tensor outputs

_PROG_CACHE = {}


def _mm(nc, out, lhsT, rhs, start=True, stop=True):
    nc.tensor.matmul(out, lhsT, rhs, start=start, stop=stop)


def build_program():
    nc = bacc.Bacc("TRN2", target_bir_lowering=False, debug=False,
                   num_devices=NCORES)

    # ---------------- DRAM I/O ----------------
    dI = lambda name, shape, dt=F32: nc.dram_tensor(name, shape, dt, kind="ExternalInput")
    headT = dI("headT", [E, B], BF16)                  # full-B head_vector^T
    relT = dI("relT", [E, BSH], BF16)                  # sharded relation_vector^T
    tailT = dI("tailT", [E, NPAD], BF16)               # sharded+padded tail_vector^T
    relidx = dI("relidx", [BSH, 2], I32)
    cbT = dI("cbT", [2 * R2, C * C // 2], PC_DT)          # tanh-input codebook, per-rel [d,c] flat
    core2 = dI("core2", [C, C * C], WM_DT)       # core reshaped [e, (c,d)]

    hsw1 = dI("hsw1", [E, E], BF16); hsb1 = dI("hsb1", [128, 4])
    hsw2 = dI("hsw2", [E, C], BF16)
    rsw1 = dI("rsw1", [E, E], BF16); rsb1 = dI("rsb1", [128, 4])
    rsw2 = dI("rsw2", [E, C], BF16); rsb2 = dI("rsb2", [128, 1])
    tsw1 = dI("tsw1", [E, E], BF16); tsb1 = dI("tsb1", [128, 4])
    tsw2 = dI("tsw2", [E, C], BF16); tsb2 = dI("tsb2", [128, 1])
    taw1 = dI("taw1", [E, E], BF16); tab1 = dI("tab1", [128, 4])
    taw2 = dI("taw2", [E, C], BF16); tab2 = dI("tab2", [128, 1])
    hrw1 = dI("hrw1", [2 * E, 2 * C], BF16); hrb1 = dI("hrb1", [128, 2])
    hrw2 = dI("hrw2", [2 * C, 2 * C], BF16); hrb2 = dI("hrb2", [128, 2])
    hrw3 = dI("hrw3", [2 * C, C], BF16); hrb3 = dI("hrb3", [128, 1])
    bn0g = dI("bn0g", [128, 1]); bn0b = dI("bn0b", [128, 1])
    bn1g = dI("bn1g", [128, 1]); bn1b = dI("bn1b", [128, 1])

    out_dt = BF16 if OUT_BF16 else F32
    tucker = nc.dram_tensor("tucker", [B, NSH], out_dt, kind="ExternalOutput")
    poss = nc.dram_tensor("poss", [B, NSH], out_dt, kind="ExternalOutput")
    dbg = {}
    if DEBUG:
        for nm, shp in [("d_hsT", [128, B]), ("d_rsT", [128, BSH]),
                        ("d_hraT", [128, BSH]), ("d_tsT", [128, NPAD]),
                        ("d_tamT", [128, NPAD]), ("d_WmTsh", [128, BSH]),
                        ("d_intTsh", [128, BSH]), ("d_WmTall", [128, B]),
                        ("d_intTall", [128, B]), ("d_ha", [128, BSH]),
                        ("d_hrm", [128, BSH]), ("d_ta0", [128, 128])]:
            dbg[nm] = nc.dram_tensor(nm, shp, F32, kind="ExternalOutput")

    with tile.TileContext(nc) as tc:
        with (
            tc.tile_pool(name="const", bufs=1) as constp,
            tc.tile_pool(name="w1p", bufs=8) as w1p,
            tc.tile_pool(name="w2p", bufs=1) as w2p,
            tc.tile_pool(name="big", bufs=2) as bigp,
            tc.tile_pool(name="xt", bufs=4) as xtp,
            tc.tile_pool(name="h1", bufs=6) as h1p,
            tc.tile_pool(name="pers", bufs=1) as pers,
            tc.tile_pool(name="small", bufs=2) as smallp,
            tc.tile_pool(name="stage", bufs=4) as stagep,
            tc.tile_pool(name="ps", bufs=6, space="PSUM") as psp,
            tc.tile_pool(name="pt", bufs=2, space="PSUM") as ptp,
            tc.tile_pool(name="dram", bufs=1, space="DRAM") as dramp,
        ):
            ident = constp.tile([128, 128], F32)
            make_identity(nc, ident[:])

            def load_const(dram_t, shape, tag, dt=F32):
                t = constp.tile(shape, dt, tag=tag)
                nc.sync.dma_start(out=t[:], in_=dram_t[:])
                return t

            hsb1_s = load_const(hsb1, [128, 4], "c01")
            rsb1_s = load_const(rsb1, [128, 4], "c02")
            tsb1_s = load_const(tsb1, [128, 4], "c03")
            tab1_s = load_const(tab1, [128, 4], "c04")
            hrb1_s = load_const(hrb1, [128, 2], "c05")
            hrb2_s = load_const(hrb2, [128, 2], "c06")
            hrb3_s = load_const(hrb3, [128, 1], "c07")
            rsb2_s = load_const(rsb2, [128, 1], "c08")
            tsb2_s = load_const(tsb2, [128, 1], "c09")
            tab2_s = load_const(tab2, [128, 1], "c10")
            bn0g_s = load_const(bn0g, [128, 1], "c11")
            bn0b_s = load_const(bn0b, [128, 1], "c12")
            bn1g_s = load_const(bn1g, [128, 1], "c13")
            bn1b_s = load_const(bn1b, [128, 1], "c14")

            # w2 weights: [512,128] viewed as [128 part, 4 kchunk, 128 c]
            def load_w2(w, tag):
                t = w2p.tile([128, 4, 128], BF16, tag=tag)
                nc.sync.dma_start(
                    out=t[:], in_=w[:].rearrange("(k p) c -> p k c", p=128))
                return t

            hsw2_s = load_w2(hsw2, "w2a")
            rsw2_s = load_w2(rsw2, "w2b")
            tsw2_s = load_w2(tsw2, "w2c")
            taw2_s = load_w2(taw2, "w2d")
            hrw3_s = w2p.tile([128, 2, 128], BF16, tag="w2e")
            nc.sync.dma_start(out=hrw3_s[:],
                              in_=hrw3[:].rearrange("(k p) c -> p k c", p=128))
            hrw2_s = w2p.tile([128, 2, 256], BF16, tag="w2f")
            nc.sync.dma_start(out=hrw2_s[:],
                              in_=hrw2[:].rearrange("(k p) c -> p k c", p=128))

            # core2 for the Wm matmuls, split into two c-halves sharing pool slots
            HALF = C * C // 2
            core2_h = []
            for h in range(2):
                ct = bigp.tile([128, HALF], WM_DT, tag="big")
                nc.sync.dma_start(out=ct[:], in_=core2[:, h * HALF:(h + 1) * HALF])
                core2_h.append(ct)

            # persistent full-B / full-shard feature tiles
            hsT_s = pers.tile([128, B], F32)          # hs^T (pre-BN)
            tsT_s = pers.tile([128, NPAD], BF16)      # ts^T (+bias)
            tamT_s = pers.tile([128, NPAD], BF16)     # tam^T
            WmT_all = pers.tile([128, B], F32)        # gathered Wm^T raw (f32 for stats)
            intT_all = pers.tile([128, B], F32)       # gathered inter^T
            WmT_nb = pers.tile([128, B], BF16)        # BN1-applied, score lhsT
            intT_nb = pers.tile([128, B], BF16)       # score lhsT

            def load_w1(w1_dram, nk):
                w1_t = []
                for k in range(nk):
                    wt = w1p.tile([128, w1_dram.shape[1]], BF16, tag="w1")
                    nc.sync.dma_start(out=wt[:], in_=w1_dram[k * 128:(k + 1) * 128, :])
                    w1_t.append(wt)
                return w1_t

            # ---------- generic 2-layer MLP producing outT [c, nb] ----------
            def load_xt(xT_dram, x_col0, nb, nk):
                xt_t = []
                for k in range(nk):
                    xt = xtp.tile([128, nb], BF16, tag="xt")
                    nc.sync.dma_start(
                        out=xt[:], in_=xT_dram[k * 128:(k + 1) * 128,
                                               x_col0:x_col0 + nb])
                    xt_t.append(xt)
                return xt_t

            def mlp2_T(w1_t, b1_tile, w2_tile, xt_t, nb, out_ap, b2_tile):
                """out_ap [128, nb] (SBUF) = (relu(x@w1+b1)@w2 (+b2))^T for nb<=512 cols."""
                w1_nk = len(w1_t)
                nm = w1_t[0].shape[1] // 128
                h1_t = []
                for m in range(nm):
                    ps = psp.tile([128, nb], F32, tag="ps")
                    for k in range(w1_nk):
                        _mm(nc, ps[:], w1_t[k][:, m * 128:(m + 1) * 128], xt_t[k][:],
                            start=(k == 0), stop=(k == w1_nk - 1))
                    h1 = h1p.tile([128, nb], BF16, tag="h1")
                    nc.scalar.activation(h1[:], ps[:], AF.Relu,
                                         bias=b1_tile[:, m:m + 1])
                    h1_t.append(h1)
                ps2 = psp.tile([128, nb], F32, tag="ps")
                for m in range(nm):
                    _mm(nc, ps2[:], w2_tile[:, m, :], h1_t[m][:],
                        start=(m == 0), stop=(m == nm - 1))
                if b2_tile is None:
                    nc.any.tensor_copy(out_ap, ps2[:])
                else:
                    nc.vector.tensor_scalar_add(out_ap, ps2[:], b2_tile[:, 0:1])
                return h1_t

            # ---------------- head MLP, full B ----------------
            hsw1_t = load_w1(hsw1, 4)
            for bg in range(B // 512):
                xt_bg = load_xt(headT, bg * 512, 512, 4)
                mlp2_T(hsw1_t, hsb1_s, hsw2_s, xt_bg, 512,
                       hsT_s[:, bg * 512:(bg + 1) * 512], None)

            # ---------------- BN0 stats (full B, local) ----------------
            def bn_scale_shift(xT_ap, nfree, g_tile, b_tile):
                nchunk = nfree // 512
                st = smallp.tile([128, nchunk, 6], F32, tag="sm6")
                for i in range(nchunk):
                    nc.vector.bn_stats(st[:, i, :], xT_ap[:, i * 512:(i + 1) * 512])
                mv = smallp.tile([128, 2], F32, tag="sm2")
                nc.vector.bn_aggr(mv[:], st[:])
                scale = smallp.tile([128, 1], F32, tag="sm1a")
                shift = smallp.tile([128, 1], F32, tag="sm1b")
                tmp = smallp.tile([128, 1], F32, tag="sm1c")
                nc.vector.tensor_scalar_add(tmp[:], mv[:, 1:2], 1e-5)
                nc.scalar.activation(scale[:], tmp[:], AF.Sqrt)
                nc.vector.reciprocal(scale[:], scale[:])
                nc.vector.tensor_mul(scale[:], scale[:], g_tile[:, 0:1])
                nc.vector.tensor_mul(tmp[:], mv[:, 0:1], scale[:])
                nc.vector.tensor_sub(shift[:], b_tile[:, 0:1], tmp[:])
                return scale, shift

            if DEBUG:
                nc.sync.dma_start(out=dbg["d_hsT"][:], in_=hsT_s[:])
            bn0_scale, bn0_shift = bn_scale_shift(hsT_s[:], B, bn0g_s, bn0b_s)

            # ha for my shard, [b, c] layout (2 tiles)
            my0 = None  # b-offset of this core's shard is data-driven via inputs;
            # the shard position is identical on every core's *own* data, so the
            # kernel uses the SAME slice index on every core: its shard was placed
            # at column block `core_id`... -- instead, host passes this core's hs
            # columns as the relT/hr inputs; for ha we need this core's 256 cols of
            # hsT. The host stages headT with *this core's* 256 columns FIRST
            # (rotated), so slice [0:256] is always "my" shard.
            haT_aff = smallp.tile([128, BSH], F32, tag="haT")
            nc.vector.tensor_scalar(haT_aff[:], hsT_s[:, 0:BSH], bn0_scale[:, 0:1],
                                    bn0_shift[:, 0:1], op0=ALU.mult, op1=ALU.add)
            ha_t = []
            for t in range(2):
                pst = ptp.tile([128, 128], F32, tag="pt")
                nc.tensor.transpose(pst[:], haT_aff[:, t * 128:(t + 1) * 128], ident[:])
                ha = smallp.tile([128, 128], F32, tag="ha")
                nc.any.tensor_copy(ha[:], pst[:])
                ha_t.append(ha)

            # ---------------- rel MLP (shard) -> rsT ----------------
            rsw1_t = load_w1(rsw1, 4)
            rsT = smallp.tile([128, BSH], F32, tag="rsT")
            xt_rel = load_xt(relT, 0, BSH, 4)
            mlp2_T(rsw1_t, rsb1_s, rsw2_s, xt_rel, BSH, rsT[:], rsb2_s)
            if DEBUG:
                nc.sync.dma_start(out=dbg["d_rsT"][:], in_=rsT[:])
            rsT_bf = smallp.tile([128, BSH], WM_DT, tag="rsTbf")
            nc.vector.tensor_copy(rsT_bf[:], rsT[:])

            # ---------------- hr MLP (shard) -> hraT -> hra -> hrm ----------------
            # layer1: input = concat(head_my, rel) => k-chunks: 4 from headT[:, :256], 4 from relT
            hr_w1 = load_w1(hrw1, 8)
            hr_x = []
            for k in range(4):
                xt = xtp.tile([128, BSH], BF16, tag="xt")
                nc.sync.dma_start(out=xt[:], in_=headT[k * 128:(k + 1) * 128, 0:BSH])
                hr_x.append(xt)
            for k in range(4):
                xt = xtp.tile([128, BSH], BF16, tag="xt")
                nc.sync.dma_start(out=xt[:], in_=relT[k * 128:(k + 1) * 128, :])
                hr_x.append(xt)
            hr_h1 = []
            for m in range(2):
                ps = psp.tile([128, BSH], F32, tag="ps")
                for k in range(8):
                    _mm(nc, ps[:], hr_w1[k][:, m * 128:(m + 1) * 128], hr_x[k][:],
                        start=(k == 0), stop=(k == 7))
                h1 = h1p.tile([128, BSH], BF16, tag="h1")
                nc.scalar.activation(h1[:], ps[:], AF.Relu, bias=hrb1_s[:, m:m + 1])
                hr_h1.append(h1)
            hr_h2 = []
            for m in range(2):
                ps = psp.tile([128, BSH], F32, tag="ps")
                for k in range(2):
                    _mm(nc, ps[:], hrw2_s[:, k, m * 128:(m + 1) * 128], hr_h1[k][:],
                        start=(k == 0), stop=(k == 1))
                h2 = h1p.tile([128, BSH], BF16, tag="h1")
                nc.scalar.activation(h2[:], ps[:], AF.Relu, bias=hrb2_s[:, m:m + 1])
                hr_h2.append(h2)
            hraT = smallp.tile([128, BSH], F32, tag="hraT")
            ps3 = psp.tile([128, BSH], F32, tag="ps")
            for k in range(2):
                _mm(nc, ps3[:], hrw3_s[:, k, :], hr_h2[k][:],
                    start=(k == 0), stop=(k == 1))
            nc.vector.tensor_scalar_add(hraT[:], ps3[:], hrb3_s[:, 0:1])

            # ---------- soft top-10 mask helper ([128,128] f32 tile) ----------
            def topk_mask_mul(x_ap, out_ap):
                """out = sigmoid((x - thr10)/TEMP) * x"""
                m8 = smallp.tile([128, 8], F32, tag="m8")
                zap = smallp.tile([128, 128], F32, tag="zap")
                nc.vector.max(out=m8[:], in_=x_ap)
                nc.vector.match_replace(out=zap[:], in_to_replace=m8[:],
                                        in_values=x_ap, imm_value=NEG)
                nc.vector.max(out=m8[:], in_=zap[:])
                thr = smallp.tile([128, 1], F32, tag="thr")
                nc.vector.tensor_scalar_mul(thr[:], m8[:, 1:2], -1.0 / TEMP)
                mask = smallp.tile([128, 128], F32, tag="mask")
                nc.scalar.activation(mask[:], x_ap, AF.Sigmoid,
                                     bias=thr[:, 0:1], scale=1.0 / TEMP)
                nc.vector.tensor_mul(out_ap, mask[:], x_ap)

            if DEBUG:
                nc.sync.dma_start(out=dbg["d_hraT"][:], in_=hraT[:])
            hrm_bf = []
            for t in range(2):
                pst = ptp.tile([128, 128], F32, tag="pt")
                nc.tensor.transpose(pst[:], hraT[:, t * 128:(t + 1) * 128], ident[:])
                hra = smallp.tile([128, 128], F32, tag="hra")
                nc.any.tensor_copy(hra[:], pst[:])
                hb = smallp.tile([128, 128], PC_DT, tag="hrmbf")
                topk_mask_mul(hra[:], hb[:])
                if DEBUG:
                    hbf = smallp.tile([128, 128], F32, tag="dbghrm")
                    nc.vector.tensor_copy(hbf[:], hb[:])
                    nc.sync.dma_start(out=dbg["d_hrm"][:, t * 128:(t + 1) * 128][:],
                                      in_=hbf[:])
                    nc.sync.dma_start(out=dbg["d_ha"][:, t * 128:(t + 1) * 128][:],
                                      in_=ha_t[t][:])
                hrm_bf.append(hb)

            # ---------------- Wm (shard): V blocks + stt accumulation ----------------
            WmT_sh = smallp.tile([128, BSH], F32, tag="WmTsh")
            for t in range(2):
                acc = smallp.tile([128, 128], F32, tag="wacc")
                for blk in range(C * C // 512):
                    hsel, hblk = divmod(blk, 16)
                    ps = psp.tile([128, 512], F32, tag="ps")
                    nc.tensor.matmul(ps[:], rsT_bf[:, t * 128:(t + 1) * 128],
                                     core2_h[hsel][:, hblk * 512:(hblk + 1) * 512],
                                     start=True, stop=True)
                    for j in range(4):
                        cidx = blk * 4 + j
                        if cidx == 0:
                            nc.vector.tensor_scalar(
                                acc[:], ps[:, j * 128:(j + 1) * 128],
                                ha_t[t][:, 0:1], None, op0=ALU.mult)
                        else:
                            nc.vector.scalar_tensor_tensor(
                                acc[:], ps[:, j * 128:(j + 1) * 128],
                                ha_t[t][:, cidx:cidx + 1], acc[:],
                                op0=ALU.mult, op1=ALU.add)
                pst = ptp.tile([128, 128], F32, tag="pt")
                nc.tensor.transpose(pst[:], acc[:], ident[:])
                nc.any.tensor_copy(WmT_sh[:, t * 128:(t + 1) * 128], pst[:])

            # ---------------- inter (shard): codebook gather + mul-reduce ----------------
            intT_sh = smallp.tile([128, BSH], F32, tag="intTsh")
            for t in range(2):
                idx = smallp.tile([128, 2], I32, tag="idx")
                nc.sync.dma_start(out=idx[:], in_=relidx[t * 128:(t + 1) * 128, :])
                intr = smallp.tile([128, 128], F32, tag="intr")
                for h in range(2):
                    pc = bigp.tile([128, HALF], PC_DT, tag="big")
                    nc.gpsimd.indirect_dma_start(
                        out=pc[:], out_offset=None, in_=cbT[:],
                        in_offset=IndirectOffsetOnAxis(ap=idx[:, h:h + 1], axis=0))
                    nc.scalar.activation(pc[:], pc[:], AF.Tanh)
                    pc3 = pc[:].rearrange("p (d c) -> p d c", c=128)
                    nc.gpsimd.tensor_tensor(
                        out=pc3, in0=pc3,
                        in1=hrm_bf[t][:, None, :].to_broadcast([128, 64, 128]),
                        op=ALU.mult)
                    nc.vector.tensor_reduce(intr[:, h * 64:(h + 1) * 64], pc3,
                                            axis=AX.X, op=ALU.add)
                pst = ptp.tile([128, 128], F32, tag="pt")
                nc.tensor.transpose(pst[:], intr[:], ident[:])
                nc.any.tensor_copy(intT_sh[:, t * 128:(t + 1) * 128], pst[:])

            if DEBUG:
                nc.sync.dma_start(out=dbg["d_WmTsh"][:], in_=WmT_sh[:])
                nc.sync.dma_start(out=dbg["d_intTsh"][:], in_=intT_sh[:])
            # ---------------- AllGather of [WmT_sh ; intT_sh] ----------------
            ag_in = dramp.tile([2, 128, BSH], F32)
            ag_out = dramp.tile([NCORES, 2, 128, BSH], F32, addr_space="Shared")
            nc.sync.dma_start(out=ag_in[0], in_=WmT_sh[:])
            nc.sync.dma_start(out=ag_in[1], in_=intT_sh[:])
            nc.gpsimd.collective_compute(
                "AllGather", ALU.bypass,
                replica_groups=[list(range(NCORES))],
                ins=[ag_in.opt()], outs=[ag_out.opt()])
            nc.sync.dma_start(
                out=WmT_all[:],
                in_=ag_out[:, 0].rearrange("r d b -> d r b"))
            nc.sync.dma_start(
                out=intT_all[:],
                in_=ag_out[:, 1].rearrange("r d b -> d r b"))

            if DEBUG:
                nc.sync.dma_start(out=dbg["d_WmTall"][:], in_=WmT_all[:])
                nc.sync.dma_start(out=dbg["d_intTall"][:], in_=intT_all[:])
            # BN1 on gathered WmT (full B), in place
            bn1_scale, bn1_shift = bn_scale_shift(WmT_all[:], B, bn1g_s, bn1b_s)
            nc.vector.tensor_scalar(WmT_nb[:], WmT_all[:], bn1_scale[:, 0:1],
                                    bn1_shift[:, 0:1], op0=ALU.mult, op1=ALU.add)
            nc.vector.tensor_copy(intT_nb[:], intT_all[:])

            # ------- tail MLPs (shard, 5 groups of 512) + interleaved scores -------
            tsw1_t = load_w1(tsw1, 4)
            taw1_t = load_w1(taw1, 4)
            evac_i = 0

            def evac(out_ap, ps_ap):
                # route 3 of 4 psum evacuations to DVE, 1 to ACT
                nonlocal evac_i
                evac_i += 1
                if evac_i % 4 == 0:
                    nc.scalar.activation(out_ap, ps_ap, AF.Copy)
                else:
                    nc.vector.tensor_copy(out_ap, ps_ap)

            for g in range(NG):
                xt_g = load_xt(tailT, g * 512, 512, 4)
                mlp2_T(tsw1_t, tsb1_s, tsw2_s, xt_g, 512,
                       tsT_s[:, g * 512:(g + 1) * 512], tsb2_s)
                taT_g = stagep.tile([128, 512], F32, tag="taT")
                mlp2_T(taw1_t, tab1_s, taw2_s, xt_g, 512,
                       taT_g[:], tab2_s)
                for j in range(4):
                    pst = ptp.tile([128, 128], F32, tag="pt")
                    nc.tensor.transpose(pst[:], taT_g[:, j * 128:(j + 1) * 128],
                                        ident[:])
                    ta_nt = smallp.tile([128, 128], F32, tag="tant")
                    nc.any.tensor_copy(ta_nt[:], pst[:])
                    if DEBUG and g == 0 and j == 0:
                        nc.sync.dma_start(out=dbg["d_ta0"][:], in_=ta_nt[:])
                    tam_nt = smallp.tile([128, 128], F32, tag="tamnt")
                    topk_mask_mul(ta_nt[:], tam_nt[:])
                    pst2 = ptp.tile([128, 128], F32, tag="pt")
                    nc.tensor.transpose(pst2[:], tam_nt[:], ident[:])
                    nc.any.tensor_copy(
                        tamT_s[:, g * 512 + j * 128:g * 512 + (j + 1) * 128],
                        pst2[:])
                # ---- scores for this n-group, all 16 b-tiles, both branches ----
                w = 512 if g < NG - 1 else NSH - (NG - 1) * 512
                for bt in range(NB_FULL):
                    ps_t = psp.tile([128, 512], F32, tag="ps")
                    _mm(nc, ps_t[:], WmT_nb[:, bt * 128:(bt + 1) * 128],
                        tsT_s[:, g * 512:(g + 1) * 512])
                    st = stagep.tile([128, 512], out_dt, tag="sst")
                    evac(st[:], ps_t[:])
                    nc.sync.dma_start(
                        out=tucker[bt * 128:(bt + 1) * 128,
                                   g * 512:g * 512 + w],
                        in_=st[:, 0:w])
                    ps_p = psp.tile([128, 512], F32, tag="ps")
                    _mm(nc, ps_p[:], intT_nb[:, bt * 128:(bt + 1) * 128],
                        tamT_s[:, g * 512:(g + 1) * 512])
                    sp = stagep.tile([128, 512], out_dt, tag="sst")
                    evac(sp[:], ps_p[:])
                    nc.sync.dma_start(
                        out=poss[bt * 128:(bt + 1) * 128,
                                 g * 512:g * 512 + w],
                        in_=sp[:, 0:w])

            if DEBUG:
                dts = smallp.tile([128, NPAD], F32, tag="dbgts")
                nc.vector.tensor_copy(dts[:], tsT_s[:])
                nc.sync.dma_start(out=dbg["d_tsT"][:], in_=dts[:])
                dtam = smallp.tile([128, NPAD], F32, tag="dbgtam")
                nc.vector.tensor_copy(dtam[:], tamT_s[:])
                nc.sync.dma_start(out=dbg["d_tamT"][:], in_=dtam[:])
    nc.finalize()
    return nc


# ---------------------------------------------------------------------------
# host side
# ---------------------------------------------------------------------------

def _to_np(x, dt=np.float32):
    return np.ascontiguousarray(np.asarray(x), dtype=dt)


def prepare_in_maps(inputs):
    head = _to_np(inputs["head_vector"])        # [B, E]
    rel = _to_np(inputs["relation_vector"])     # [B, E]
    ridx = np.ascontiguousarray(np.asarray(inputs["relation_index"]).astype(np.int32))
    tailv = _to_np(inputs["tail_vector"])       # [N, E]
    codebook = _to_np(inputs["codebook"])       # [R2, C, C]
    core = _to_np(inputs["core"])               # [C, C, C]

    pc_np = np.dtype(ml_dtypes.bfloat16) if PC_DT == BF16 else np.float32
    wm_np = np.dtype(ml_dtypes.bfloat16) if WM_DT == BF16 else np.float32

    # per-relation matrices transposed to [d, c] then flattened; tanh on device
    cbT_host = np.ascontiguousarray(
        codebook.transpose(0, 2, 1).reshape(2 * R2, C * C // 2)).astype(pc_np)
    core2_host = np.ascontiguousarray(core.reshape(C, C * C)).astype(wm_np)

    bf = np.dtype(ml_dtypes.bfloat16)
    headT = np.ascontiguousarray(head.T).astype(bf)        # [E, B]
    relT_full = np.ascontiguousarray(rel.T).astype(bf)     # [E, B]
    tailT_full = np.ascontiguousarray(tailv.T).astype(bf)  # [E, N]

    def chunked_bias(b, nk):
        return np.ascontiguousarray(_to_np(b).reshape(nk, 128).T)

    wcast = lambda k: _to_np(inputs[k]).astype(bf)
    weights_common = {
        "hsw1": wcast("hsw1"), "hsb1": chunked_bias(inputs["hsb1"], 4),
        "hsw2": wcast("hsw2"),
        "rsw1": wcast("rsw1"), "rsb1": chunked_bias(inputs["rsb1"], 4),
        "rsw2": wcast("rsw2"), "rsb2": _to_np(inputs["rsb2"]).reshape(128, 1),
        "tsw1": wcast("tsw1"), "tsb1": chunked_bias(inputs["tsb1"], 4),
        "tsw2": wcast("tsw2"), "tsb2": _to_np(inputs["tsb2"]).reshape(128, 1),
        "taw1": wcast("taw1"), "tab1": chunked_bias(inputs["tab1"], 4),
        "taw2": wcast("taw2"), "tab2": _to_np(inputs["tab2"]).reshape(128, 1),
        "hrw1": wcast("hrw1"), "hrb1": chunked_bias(inputs["hrb1"], 2),
        "hrw2": wcast("hrw2"), "hrb2": chunked_bias(inputs["hrb2"], 2),
        "hrw3": wcast("hrw3"), "hrb3": _to_np(inputs["hrb3"]).reshape(128, 1),
        "bn0g": _to_np(inputs["bn0_g"]).reshape(128, 1),
        "bn0b": _to_np(inputs["bn0_b"]).reshape(128, 1),
        "bn1g": _to_np(inputs["bn1_g"]).reshape(128, 1),
        "bn1b": _to_np(inputs["bn1_b"]).reshape(128, 1),
        "cbT": cbT_host, "core2": core2_host,
    }

    in_maps = []
    for k in range(NCORES):
        b0 = k * BSH
        n0 = k * NSH
        # rotate headT so THIS core's 256 b-columns come first; BN0 stats are
        # order-invariant, and slices [0:256] are "my" shard on every core.
        headT_k = np.ascontiguousarray(np.roll(headT, -b0, axis=1))
        tailT_k = np.zeros((E, NPAD), bf)
        tailT_k[:, :NSH] = tailT_full[:, n0:n0 + NSH]
        m = dict(weights_common)
        m["headT"] = headT_k
        m["relT"] = np.ascontiguousarray(relT_full[:, b0:b0 + BSH])
        m["tailT"] = tailT_k
        ri = ridx[b0:b0 + BSH].astype(np.int32)
        m["relidx"] = np.ascontiguousarray(
            np.stack([2 * ri, 2 * ri + 1], axis=1))
        in_maps.append(m)
    return in_maps


def assemble_outputs(results):
    tuckers, posses = [], []
    for k in range(NCORES):
        r = results[k]
        # rows already in true b order: the AllGather is rank-ordered and each
        # rank's shard is its true b-block (the headT rotation only affects the
        # internal hsT column order, whose consumers are order-invariant).
        tuckers.append(np.asarray(r["tucker"]).astype(np.float32))
        posses.append(np.asarray(r["poss"]).astype(np.float32))
    tucker_full = np.concatenate(tuckers, axis=1)
    poss_full = np.concatenate(posses, axis=1)
    return tucker_full, poss_full


def kernel(**inputs):
    if "prog" not in _PROG_CACHE:
        _PROG_CACHE["prog"] = build_program()
    nc = _PROG_CACHE["prog"]
    in_maps = prepare_in_maps(inputs)
    res = run_bass_kernel_spmd(nc, in_maps, list(range(NCORES)))
    return assemble_outputs(res.results)



# revision 11
# speedup vs baseline: 1.0302x; 1.0302x over previous
"""Trainium2 Bass kernel for nn_BaseModel_74302934220896 (TuckER + possibility-codebook).

Contract: kernel(**inputs) takes FULL unsharded inputs (as in reference.setup_inputs())
and returns the full output tuple (tucker_logits [B,N] f32, possibility_score [B,N] f32).

Sharding (8 cores):
  - B (2048) -> 8 x 256 for head/relation/hr/codebook-gather paths
  - N (20000) -> 8 x 2500 (padded to 2560) for tail features and the [B,N] score matmuls
  - BN0 statistics via a tiny AllGather of per-rank bn_stats
  - ONE main AllGather carries the per-core [WmT(raw, bf16); interT(bf16)] shards;
    BN1 statistics computed locally from the gathered full-B WmT.

v2 design notes:
  - all MLP biases folded into k=1 "bias-row" matmuls (no separate bias adds)
  - Wm / inter einsums ('bc,bcd->bd') via broadcast tensor_tensor on GPSIMD
    (in-place, bf16) + wide last-axis tensor_reduce on DVE (bf16 accumulate)
  - scores evacuated from PSUM in [128,1024] chunks, round-robin DVE/ACT
  - bf16 outputs (host upcasts to f32)
"""

import sys

sys.path.insert(0, "/opt/trn_rl_repo")

import numpy as np
import ml_dtypes

import concourse.bass as bass
import concourse.bacc as bacc
import concourse.mybir as mybir
import concourse.tile as tile
from concourse.bass import IndirectOffsetOnAxis
from concourse.bass_utils import run_bass_kernel_spmd
from concourse.masks import make_identity

F32 = mybir.dt.float32
BF16 = mybir.dt.bfloat16
I32 = mybir.dt.int32
AF = mybir.ActivationFunctionType
ALU = mybir.AluOpType
AX = mybir.AxisListType

B, N, E, C, R2 = 2048, 20000, 512, 128, 474
NCORES = 8
BSH = B // NCORES            # 256 b rows per core
NSH = N // NCORES            # 2500 tail rows per core
NPAD = 2560                  # padded to 5 groups of 512
NG = NPAD // 512             # 5 n-groups
NB_FULL = B // 128           # 16 b-tiles over full B
TEMP = 0.5
NEG = -1.0e30

_PROG_CACHE = {}


def build_program():
    nc = bacc.Bacc("TRN2", target_bir_lowering=False, debug=False,
                   num_devices=NCORES)

    # ---------------- DRAM I/O ----------------
    dI = lambda name, shape, dt=BF16: nc.dram_tensor(name, shape, dt, kind="ExternalInput")
    headT = dI("headT", [E, BSH])                  # sharded head_vector^T
    relT = dI("relT", [E, BSH])                    # sharded relation_vector^T
    tailT = dI("tailT", [E, NPAD])                 # sharded+padded tail_vector^T
    relidx = dI("relidx", [BSH, 2], I32)
    cbT = dI("cbT", [2 * R2, C * C // 2])          # tanh-input codebook, rows (d-half, c)
    core_dc = dI("core_dc", [C, C * C])            # core as [e, (d, c)] (c fastest)

    hsw1 = dI("hsw1", [E, E]); rsw1 = dI("rsw1", [E, E])
    tsw1 = dI("tsw1", [E, E]); taw1 = dI("taw1", [E, E])
    hsw2 = dI("hsw2", [E, C]); rsw2 = dI("rsw2", [E, C])
    tsw2 = dI("tsw2", [E, C]); taw2 = dI("taw2", [E, C])
    hrw1 = dI("hrw1", [2 * E, 2 * C])
    hrw2 = dI("hrw2", [2 * C, 2 * C])
    hrw3 = dI("hrw3", [2 * C, C])
    # bias rows [1, dim]
    hsb1 = dI("hsb1", [1, E]); hsb2 = dI("hsb2", [1, C])
    rsb1 = dI("rsb1", [1, E]); rsb2 = dI("rsb2", [1, C])
    tsb1 = dI("tsb1", [1, E]); tsb2 = dI("tsb2", [1, C])
    tab1 = dI("tab1", [1, E]); tab2 = dI("tab2", [1, C])
    hrb1 = dI("hrb1", [1, 2 * C]); hrb2 = dI("hrb2", [1, 2 * C]); hrb3 = dI("hrb3", [1, C])
    bn0g = dI("bn0g", [128, 1], F32); bn0b = dI("bn0b", [128, 1], F32)
    bn1g = dI("bn1g", [128, 1], F32); bn1b = dI("bn1b", [128, 1], F32)

    tucker = nc.dram_tensor("tucker", [B, NSH], BF16, kind="ExternalOutput")
    poss = nc.dram_tensor("poss", [B, NSH], BF16, kind="ExternalOutput")

    with tile.TileContext(nc) as tc:
        with (
            tc.tile_pool(name="const", bufs=1) as constp,
            tc.tile_pool(name="w1p", bufs=4) as w1p,         # 4 bufs per family tag
            tc.tile_pool(name="hrw1p", bufs=8) as hrw1p,
            tc.tile_pool(name="w2p", bufs=1) as w2p,
            tc.tile_pool(name="big", bufs=1) as bigp,        # core_dc halves (2 tags)
            tc.tile_pool(name="pcp", bufs=2) as pcp,         # gathered codebook tiles
            tc.tile_pool(name="xt", bufs=8) as xtp,
            tc.tile_pool(name="h1", bufs=2) as h1p,
            tc.tile_pool(name="wsb", bufs=2) as wsbp,        # W 4-chunk sbuf tiles
            tc.tile_pool(name="pers", bufs=1) as pers,
            tc.tile_pool(name="small", bufs=2) as smallp,
            tc.tile_pool(name="stage", bufs=3) as stagep,
            tc.tile_pool(name="psA", bufs=2, space="PSUM") as psA,   # [128,2,512] = 2 banks
            tc.tile_pool(name="psB", bufs=2, space="PSUM") as psB,   # [128,512]   = 1 bank
            tc.tile_pool(name="psC", bufs=1, space="PSUM") as psC,   # [128,128]
            tc.tile_pool(name="dram", bufs=1, space="DRAM") as dramp,
        ):
            # ---------------- constants ----------------
            ident = constp.tile([128, 128], F32)
            make_identity(nc, ident[:])
            ident_bf = constp.tile([128, 128], BF16, tag="idbf")
            nc.gpsimd.tensor_copy(ident_bf[:], ident[:])
            ones_row = constp.tile([1, 512], BF16, tag="ones")
            nc.gpsimd.memset(ones_row[:], 1.0)

            def load_const(dram_t, shape, tag, dt=F32):
                t = constp.tile(shape, dt, tag=tag)
                nc.sync.dma_start(out=t[:], in_=dram_t[:])
                return t

            bias = {}
            for nm, dr, dim in [("hsb1", hsb1, E), ("hsb2", hsb2, C),
                                ("rsb1", rsb1, E), ("rsb2", rsb2, C),
                                ("tsb1", tsb1, E), ("tsb2", tsb2, C),
                                ("tab1", tab1, E), ("tab2", tab2, C),
                                ("hrb1", hrb1, 2 * C), ("hrb2", hrb2, 2 * C),
                                ("hrb3", hrb3, C)]:
                bias[nm] = load_const(dr, [1, dim], "b_" + nm, BF16)
            bn0g_s = load_const(bn0g, [128, 1], "c11")
            bn0b_s = load_const(bn0b, [128, 1], "c12")
            bn1g_s = load_const(bn1g, [128, 1], "c13")
            bn1b_s = load_const(bn1b, [128, 1], "c14")

            # w1 weights: k-chunk tiles [128, mdim]
            def load_w1(w1_dram, nk, tag="w1", pool=None):
                pool = pool or w1p
                w1_t = []
                for k in range(nk):
                    wt = pool.tile([128, w1_dram.shape[1]], BF16, tag=tag)
                    nc.sync.dma_start(out=wt[:], in_=w1_dram[k * 128:(k + 1) * 128, :])
                    w1_t.append(wt)
                return w1_t

            # w2 weights: [kdim,128] viewed as [128 part, nk kchunk, 128 c]
            def load_w2(w, nk, tag):
                t = w2p.tile([128, nk, w.shape[1]], BF16, tag=tag)
                nc.sync.dma_start(
                    out=t[:], in_=w[:].rearrange("(k p) c -> p k c", p=128))
                return t

            hsw2_s = load_w2(hsw2, 4, "w2a")
            rsw2_s = load_w2(rsw2, 4, "w2b")
            tsw2_s = load_w2(tsw2, 4, "w2c")
            taw2_s = load_w2(taw2, 4, "w2d")
            hrw2_s = load_w2(hrw2, 2, "w2f")
            hrw3_s = load_w2(hrw3, 2, "w2e")

            # core_dc in SBUF, two halves [128, 8192]
            HALF = C * C // 2
            core_h = []
            for h in range(2):
                ct = bigp.tile([128, HALF], BF16, tag=f"core{h}", name=f"core{h}")
                nc.sync.dma_start(out=ct[:], in_=core_dc[:, h * HALF:(h + 1) * HALF])
                core_h.append(ct)

            # persistent tiles
            tsT_s = pers.tile([128, NPAD], BF16)      # ts^T (+biases)
            tamT_s = pers.tile([128, NPAD], BF16)     # tam^T
            WmT_all = pers.tile([128, B], BF16)       # gathered Wm^T raw
            intT_all = pers.tile([128, B], BF16)      # gathered inter^T
            WmT_nb = pers.tile([128, B], BF16)        # BN1-applied, score lhsT
            hsT_sh = pers.tile([128, BSH], F32)       # hs^T shard (pre-BN)
            rsT_bf = pers.tile([128, BSH], BF16)
            WmT_sh = pers.tile([128, BSH], BF16)
            intT_sh = pers.tile([128, BSH], BF16)

            # ---------------- engine routing ----------------
            evac_i = [0]

            def evac(out_ap, ps_ap, which=None):
                """PSUM->SBUF copy routed round-robin DVE/ACT."""
                if which is None:
                    evac_i[0] += 1
                    which = "v" if evac_i[0] % 2 else "a"
                if which == "v":
                    nc.vector.tensor_copy(out_ap, ps_ap)
                else:
                    nc.scalar.activation(out_ap, ps_ap, AF.Copy)

            relu_i = [0]

            def relu_evac(out_ap, ps_ap):
                relu_i[0] += 1
                if relu_i[0] % 2:
                    nc.vector.tensor_relu(out_ap, ps_ap)
                else:
                    nc.scalar.activation(out_ap, ps_ap, AF.Relu)

            def mm(out, lhsT, rhs, start, stop):
                nc.tensor.matmul(out, lhsT, rhs, start=start, stop=stop)

            # ---------------- MLP helpers ----------------
            def load_xt(xT_dram, col0, nb, nk):
                xt_t = []
                for k in range(nk):
                    xt = xtp.tile([128, nb], BF16, tag=f"xt{nb}")
                    nc.sync.dma_start(
                        out=xt[:], in_=xT_dram[k * 128:(k + 1) * 128, col0:col0 + nb])
                    xt_t.append(xt)
                return xt_t

            def mlp_l1(w1_t, b1row, xt_t, nb, nm=4):
                """h1 [128, nm, nb] bf16 = relu(x @ w1 + b1)^T in m-chunks."""
                h1 = h1p.tile([128, nm, nb], BF16, tag=f"h1_{nm}_{nb}")
                for half in range(nm // 2):
                    ps = psA.tile([128, 2, nb], F32, tag="A")
                    for mi in range(2):
                        m = half * 2 + mi
                        for k in range(len(w1_t)):
                            mm(ps[:, mi, :], w1_t[k][:, m * 128:(m + 1) * 128],
                               xt_t[k][:], start=(k == 0), stop=False)
                        mm(ps[:, mi, :], b1row[0:1, m * 128:(m + 1) * 128],
                           ones_row[0:1, 0:nb], start=False, stop=True)
                    relu_evac(h1[:, half * 2:half * 2 + 2, :], ps[:])
                return h1

            def mlp_l2_T(w2_s, b2row, h1, nb, out_ap, nm=4):
                """out [128 c, nb] = (h1 @ w2 + b2)^T ; evac routed."""
                ps2 = psB.tile([128, nb], F32, tag="B")
                for m in range(nm):
                    mm(ps2[:], w2_s[:, m, :], h1[:, m, :], start=(m == 0), stop=False)
                mm(ps2[:], b2row[0:1, :], ones_row[0:1, 0:nb], start=False, stop=True)
                evac(out_ap, ps2[:])

            # ---------- soft top-10 mask ([128,128] f32 AP, may be PSUM) ----------
            def topk_mask_mul(x_ap, out_ap):
                """out = sigmoid((x - thr10)/TEMP) * x   (out bf16)"""
                m8 = smallp.tile([128, 8], F32, tag="m8")
                zap = smallp.tile([128, 128], F32, tag="zap")
                nc.vector.max(out=m8[:], in_=x_ap)
                nc.vector.match_replace(out=zap[:], in_to_replace=m8[:],
                                        in_values=x_ap, imm_value=NEG)
                nc.vector.max(out=m8[:], in_=zap[:])
                thr = smallp.tile([128, 1], F32, tag="thr")
                nc.vector.tensor_scalar_mul(thr[:], m8[:, 1:2], -1.0 / TEMP)
                mask = smallp.tile([128, 128], F32, tag="mask")
                nc.scalar.activation(mask[:], x_ap, AF.Sigmoid,
                                     bias=thr[:, 0:1], scale=1.0 / TEMP)
                nc.vector.tensor_mul(out_ap, mask[:], x_ap)

            # =========== Phase 1: head shard MLP + BN0 stats AG ===========
            xt_h = load_xt(headT, 0, BSH, 4)
            hsw1_t = load_w1(hsw1, 4, "w1hs")
            h1h = mlp_l1(hsw1_t, bias["hsb1"], xt_h, BSH)
            mlp_l2_T(hsw2_s, bias["hsb2"], h1h, BSH, hsT_sh[:])

            st0 = smallp.tile([128, 6], F32, tag="st0")
            nc.vector.bn_stats(st0[:], hsT_sh[:])
            ag1_in = dramp.tile([128, 6], F32)
            ag1_out = dramp.tile([NCORES, 128, 6], F32, addr_space="Shared")
            nc.sync.dma_start(out=ag1_in[:], in_=st0[:])
            nc.gpsimd.collective_compute(
                "AllGather", ALU.bypass,
                replica_groups=[list(range(NCORES))],
                ins=[ag1_in.opt()], outs=[ag1_out.opt()])

            # =========== rel MLP ===========
            xt_r = load_xt(relT, 0, BSH, 4)
            rsw1_t = load_w1(rsw1, 4, "w1rs")
            h1r = mlp_l1(rsw1_t, bias["rsb1"], xt_r, BSH)
            mlp_l2_T(rsw2_s, bias["rsb2"], h1r, BSH, rsT_bf[:])

            # =========== hr MLP (3 layers) ===========
            hrw1_t = load_w1(hrw1, 8, "w1hr", hrw1p)
            hr_x = xt_h + xt_r     # concat(head, rel) k-chunks
            hrh1 = h1p.tile([128, 2, BSH], BF16, tag="hrh1")
            ps = psA.tile([128, 2, BSH], F32, tag="A")
            for mi in range(2):
                for k in range(8):
                    mm(ps[:, mi, :], hrw1_t[k][:, mi * 128:(mi + 1) * 128],
                       hr_x[k][:], start=(k == 0), stop=False)
                mm(ps[:, mi, :], bias["hrb1"][0:1, mi * 128:(mi + 1) * 128],
                   ones_row[0:1, 0:BSH], start=False, stop=True)
            relu_evac(hrh1[:], ps[:])
            hrh2 = h1p.tile([128, 2, BSH], BF16, tag="hrh2")
            ps = psA.tile([128, 2, BSH], F32, tag="A")
            for mi in range(2):
                for k in range(2):
                    mm(ps[:, mi, :], hrw2_s[:, k, mi * 128:(mi + 1) * 128],
                       hrh1[:, k, :], start=(k == 0), stop=False)
                mm(ps[:, mi, :], bias["hrb2"][0:1, mi * 128:(mi + 1) * 128],
                   ones_row[0:1, 0:BSH], start=False, stop=True)
            relu_evac(hrh2[:], ps[:])
            hraT = smallp.tile([128, BSH], F32, tag="hraT")
            ps2 = psB.tile([128, BSH], F32, tag="B")
            for k in range(2):
                mm(ps2[:], hrw3_s[:, k, :], hrh2[:, k, :], start=(k == 0), stop=False)
            mm(ps2[:], bias["hrb3"][0:1, :], ones_row[0:1, 0:BSH],
               start=False, stop=True)
            evac(hraT[:], ps2[:])

            # hrm (masked hra) in [b, c] layout, bf16, per t-tile
            hrm_bf = []
            for t in range(2):
                pst = psC.tile([128, 128], F32, tag="C")
                nc.tensor.transpose(pst[:], hraT[:, t * 128:(t + 1) * 128], ident[:])
                hb = smallp.tile([128, 128], BF16, tag="hrmbf")
                topk_mask_mul(pst[:], hb[:])
                hrm_bf.append(hb)

            # =========== tail weights ===========
            tsw1_t = load_w1(tsw1, 4, "w1ts")
            taw1_t = load_w1(taw1, 4, "w1ta")

            # =========== BN0 finalize (waits on AG1) ===========
            def bn_finalize(stats_all_ap, nchunk, g_tile, b_tile):
                mv = smallp.tile([128, 2], F32, tag="mv")
                nc.vector.bn_aggr(mv[:], stats_all_ap)
                scale = smallp.tile([128, 1], F32, tag="sc")
                shift = smallp.tile([128, 1], F32, tag="sh")
                tmp = smallp.tile([128, 1], F32, tag="tm")
                nc.vector.tensor_scalar_add(tmp[:], mv[:, 1:2], 1e-5)
                nc.scalar.activation(scale[:], tmp[:], AF.Sqrt)
                nc.vector.reciprocal(scale[:], scale[:])
                nc.vector.tensor_mul(scale[:], scale[:], g_tile[:, 0:1])
                nc.vector.tensor_mul(tmp[:], mv[:, 0:1], scale[:])
                nc.vector.tensor_sub(shift[:], b_tile[:, 0:1], tmp[:])
                return scale, shift

            # Wm / inter shard accumulators in [b, d] layout
            Wm_sh = [smallp.tile([128, 128], BF16, tag=f"wmsh{t}",
                                 name=f"Wm_sh{t}") for t in range(2)]
            inter_sh = [smallp.tile([128, 128], BF16, tag=f"ish{t}",
                                    name=f"inter_sh{t}") for t in range(2)]

            idx_t = []
            for t in range(2):
                idx = smallp.tile([128, 2], I32, tag=f"idx{t}")
                nc.sync.dma_start(out=idx[:], in_=relidx[t * 128:(t + 1) * 128, :])
                idx_t.append(idx)

            # ---------------- W 4-chunk group (einsum via GPS+DVE) ----------------
            def w_group(gi):
                t, q = divmod(gi, 8)          # q = which 4-chunk quarter (d0 = 16q)
                wsb = wsbp.tile([128, 4, 4, 128], BF16, tag="wsb")
                for jj in range(4):
                    j = q * 4 + jj            # chunk index 0..31 (d = 4j..4j+3)
                    hsel, off = divmod(j * 512, HALF)
                    pw = psB.tile([128, 512], F32, tag="B")
                    mm(pw[:], rsT_bf[:, t * 128:(t + 1) * 128],
                       core_h[hsel][:, off:off + 512], start=True, stop=True)
                    evac(wsb[:, jj, :, :], pw[:].rearrange("p (d c) -> p d c", c=128))
                w4 = wsb[:].rearrange("p j d c -> p (j d) c")
                nc.gpsimd.tensor_tensor(
                    out=w4, in0=w4,
                    in1=ha_bf[t][:, None, :].to_broadcast([128, 16, 128]),
                    op=ALU.mult)
                with nc.allow_low_precision("bf16 Wm accumulate"):
                    nc.vector.tensor_reduce(Wm_sh[t][:, 16 * q:16 * (q + 1)],
                                            w4, axis=AX.X, op=ALU.add)

            # ---------------- inter tile (t, h) ----------------
            def inter_tile(t, h):
                pc = pcp.tile([128, HALF], BF16, tag="pc")
                nc.gpsimd.indirect_dma_start(
                    out=pc[:], out_offset=None, in_=cbT[:],
                    in_offset=IndirectOffsetOnAxis(ap=idx_t[t][:, h:h + 1], axis=0))
                nc.scalar.activation(pc[:], pc[:], AF.Tanh)
                pc3 = pc[:].rearrange("p (d c) -> p d c", c=128)
                nc.gpsimd.tensor_tensor(
                    out=pc3, in0=pc3,
                    in1=hrm_bf[t][:, None, :].to_broadcast([128, 64, 128]),
                    op=ALU.mult)
                with nc.allow_low_precision("bf16 inter accumulate"):
                    nc.vector.tensor_reduce(inter_sh[t][:, h * 64:(h + 1) * 64],
                                            pc3, axis=AX.X, op=ALU.add)

            # ---------------- tail MLP group ----------------
            def tail_group(g):
                xt_g = load_xt(tailT, g * 512, 512, 4)
                # ts branch -> tsT_s columns
                h1t = mlp_l1(tsw1_t, bias["tsb1"], xt_g, 512)
                mlp_l2_T(tsw2_s, bias["tsb2"], h1t, 512,
                         tsT_s[:, g * 512:(g + 1) * 512])
                # ta branch -> [n, c] tiles -> topk -> transpose -> tamT
                h1a = mlp_l1(taw1_t, bias["tab1"], xt_g, 512)
                for nt in range(4):
                    pt = psC.tile([128, 128], F32, tag="C")
                    for m in range(4):
                        mm(pt[:], h1a[:, m, nt * 128:(nt + 1) * 128],
                           taw2_s[:, m, :], start=(m == 0), stop=False)
                    mm(pt[:], ones_row[0:1, 0:128], bias["tab2"][0:1, :],
                       start=False, stop=True)
                    tam_nc = smallp.tile([128, 128], BF16, tag="tamnc")
                    topk_mask_mul(pt[:], tam_nc[:])
                    ptT = psC.tile([128, 128], BF16, tag="Cb")
                    nc.tensor.transpose(ptT[:], tam_nc[:], ident_bf[:])
                    evac(tamT_s[:, g * 512 + nt * 128:g * 512 + (nt + 1) * 128],
                         ptT[:])

            # =========== Phase 2: interleaved tail / Wm / inter ===========
            tail_group(0)

            # BN0 finalize (waits on AG1) + ha tiles, needed by w_group
            stats0_all = smallp.tile([128, NCORES, 6], F32, tag="sall")
            nc.sync.dma_start(out=stats0_all[:],
                              in_=ag1_out[:].rearrange("r p s -> p r s"))
            bn0_scale, bn0_shift = bn_finalize(stats0_all[:], NCORES, bn0g_s, bn0b_s)
            haT = smallp.tile([128, BSH], F32, tag="haT")
            nc.vector.tensor_scalar(haT[:], hsT_sh[:], bn0_scale[:, 0:1],
                                    bn0_shift[:, 0:1], op0=ALU.mult, op1=ALU.add)
            haT_bf = smallp.tile([128, BSH], BF16, tag="haTbf")
            nc.gpsimd.tensor_copy(haT_bf[:], haT[:])
            ha_bf = []
            for t in range(2):
                pst = psC.tile([128, 128], BF16, tag="Cb")
                nc.tensor.transpose(pst[:], haT_bf[:, t * 128:(t + 1) * 128],
                                    ident_bf[:])
                hb = smallp.tile([128, 128], BF16, tag="habf")
                evac(hb[:], pst[:])
                ha_bf.append(hb)

            for g in range(1, NG):
                for q in range(4):
                    w_group((g - 1) * 4 + q)
                inter_tile((g - 1) // 2, (g - 1) % 2)
                tail_group(g)

            # =========== Phase 3: pack shard, AllGather, BN1 ===========
            for t in range(2):
                pst = psC.tile([128, 128], BF16, tag="Cb")
                nc.tensor.transpose(pst[:], Wm_sh[t][:], ident_bf[:])
                evac(WmT_sh[:, t * 128:(t + 1) * 128], pst[:])
                pst2 = psC.tile([128, 128], BF16, tag="Cb")
                nc.tensor.transpose(pst2[:], inter_sh[t][:], ident_bf[:])
                evac(intT_sh[:, t * 128:(t + 1) * 128], pst2[:])

            ag2_in = dramp.tile([2, 128, BSH], BF16)
            ag2_out = dramp.tile([NCORES, 2, 128, BSH], BF16, addr_space="Shared")
            nc.sync.dma_start(out=ag2_in[0], in_=WmT_sh[:])
            nc.sync.dma_start(out=ag2_in[1], in_=intT_sh[:])
            nc.gpsimd.collective_compute(
                "AllGather", ALU.bypass,
                replica_groups=[list(range(NCORES))],
                ins=[ag2_in.opt()], outs=[ag2_out.opt()])
            nc.sync.dma_start(
                out=WmT_all[:], in_=ag2_out[:, 0].rearrange("r d b -> d r b"))
            nc.sync.dma_start(
                out=intT_all[:], in_=ag2_out[:, 1].rearrange("r d b -> d r b"))

            # BN1 on gathered WmT (full B)
            st1 = smallp.tile([128, 4, 6], F32, tag="st1")
            for i in range(4):
                nc.vector.bn_stats(st1[:, i, :], WmT_all[:, i * 512:(i + 1) * 512])
            bn1_scale, bn1_shift = bn_finalize(st1[:], 4, bn1g_s, bn1b_s)
            nc.vector.tensor_scalar(WmT_nb[:], WmT_all[:], bn1_scale[:, 0:1],
                                    bn1_shift[:, 0:1], op0=ALU.mult, op1=ALU.add)

            # =========== Phase 4: scores ===========
            NLAST = NSH - 4 * 512               # 452 valid cols in group 4
            for bt in range(NB_FULL):
                r0 = bt * 128
                for lhsT, outd in ((WmT_nb, tucker), (intT_all, poss)):
                    rhs = tsT_s if outd is tucker else tamT_s
                    for half in range(2):
                        psq = psA.tile([128, 2, 512], F32, tag="A")
                        for gg in range(2):
                            g = half * 2 + gg
                            mm(psq[:, gg, :], lhsT[:, r0:r0 + 128],
                               rhs[:, g * 512:(g + 1) * 512], start=True, stop=True)
                        stq = stagep.tile([128, 1024], BF16, tag="stq")
                        evac(stq[:], psq[:].rearrange("p g n -> p (g n)"))
                        nc.sync.dma_start(
                            out=outd[r0:r0 + 128, half * 1024:(half + 1) * 1024],
                            in_=stq[:])
                    ps4 = psB.tile([128, 512], F32, tag="B")
                    mm(ps4[:], lhsT[:, r0:r0 + 128], rhs[:, 4 * 512:5 * 512],
                       start=True, stop=True)
                    st4 = stagep.tile([128, 512], BF16, tag="st4")
                    evac(st4[:], ps4[:])
                    nc.sync.dma_start(out=outd[r0:r0 + 128, 2048:NSH],
                                      in_=st4[:, 0:NLAST])

    nc.finalize()
    return nc


# ---------------------------------------------------------------------------
# host side
# ---------------------------------------------------------------------------

BF = np.dtype(ml_dtypes.bfloat16)


def _to_np(x, dt=np.float32):
    return np.ascontiguousarray(np.asarray(x), dtype=dt)


def prepare_in_maps(inputs):
    head = _to_np(inputs["head_vector"])        # [B, E]
    rel = _to_np(inputs["relation_vector"])     # [B, E]
    ridx = np.ascontiguousarray(np.asarray(inputs["relation_index"]).astype(np.int32))
    tailv = _to_np(inputs["tail_vector"])       # [N, E]
    codebook = _to_np(inputs["codebook"])       # [R2, C, C]
    core = _to_np(inputs["core"])               # [C, C, C]

    # per-relation matrices transposed to [d, c] then flattened; tanh on device
    cbT_host = np.ascontiguousarray(
        codebook.transpose(0, 2, 1).reshape(2 * R2, C * C // 2)).astype(BF)
    # core as [e, (d, c)] with c fastest
    core_dc_host = np.ascontiguousarray(
        core.transpose(0, 2, 1).reshape(C, C * C)).astype(BF)

    headT_full = np.ascontiguousarray(head.T).astype(BF)   # [E, B]
    relT_full = np.ascontiguousarray(rel.T).astype(BF)     # [E, B]
    tailT_full = np.ascontiguousarray(tailv.T).astype(BF)  # [E, N]

    brow = lambda k: _to_np(inputs[k]).reshape(1, -1).astype(BF)
    wcast = lambda k: _to_np(inputs[k]).astype(BF)
    weights_common = {
        "hsw1": wcast("hsw1"), "hsb1": brow("hsb1"),
        "hsw2": wcast("hsw2"), "hsb2": brow("hsb2"),
        "rsw1": wcast("rsw1"), "rsb1": brow("rsb1"),
        "rsw2": wcast("rsw2"), "rsb2": brow("rsb2"),
        "tsw1": wcast("tsw1"), "tsb1": brow("tsb1"),
        "tsw2": wcast("tsw2"), "tsb2": brow("tsb2"),
        "taw1": wcast("taw1"), "tab1": brow("tab1"),
        "taw2": wcast("taw2"), "tab2": brow("tab2"),
        "hrw1": wcast("hrw1"), "hrb1": brow("hrb1"),
        "hrw2": wcast("hrw2"), "hrb2": brow("hrb2"),
        "hrw3": wcast("hrw3"), "hrb3": brow("hrb3"),
        "bn0g": _to_np(inputs["bn0_g"]).reshape(128, 1),
        "bn0b": _to_np(inputs["bn0_b"]).reshape(128, 1),
        "bn1g": _to_np(inputs["bn1_g"]).reshape(128, 1),
        "bn1b": _to_np(inputs["bn1_b"]).reshape(128, 1),
        "cbT": cbT_host, "core_dc": core_dc_host,
    }

    in_maps = []
    for k in range(NCORES):
        b0 = k * BSH
        n0 = k * NSH
        tailT_k = np.zeros((E, NPAD), BF)
        tailT_k[:, :NSH] = tailT_full[:, n0:n0 + NSH]
        m = dict(weights_common)
        m["headT"] = np.ascontiguousarray(headT_full[:, b0:b0 + BSH])
        m["relT"] = np.ascontiguousarray(relT_full[:, b0:b0 + BSH])
        m["tailT"] = tailT_k
        ri = ridx[b0:b0 + BSH]
        m["relidx"] = np.ascontiguousarray(
            np.stack([2 * ri, 2 * ri + 1], axis=1))
        in_maps.append(m)
    return in_maps


def assemble_outputs(results):
    tuckers, posses = [], []
    for k in range(NCORES):
        r = results[k]
        tuckers.append(np.asarray(r["tucker"]).astype(np.float32))
        posses.append(np.asarray(r["poss"]).astype(np.float32))
    tucker_full = np.concatenate(tuckers, axis=1)
    poss_full = np.concatenate(posses, axis=1)
    return tucker_full, poss_full


def kernel(**inputs):
    if "prog" not in _PROG_CACHE:
        _PROG_CACHE["prog"] = build_program()
    nc = _PROG_CACHE["prog"]
    in_maps = prepare_in_maps(inputs)
    res = run_bass_kernel_spmd(nc, in_maps, list(range(NCORES)))
    return assemble_outputs(res.results)


# revision 12
# speedup vs baseline: 1.2948x; 1.2569x over previous
"""Trainium2 Bass kernel for nn_BaseModel_74302934220896 (TuckER + possibility-codebook).

Contract: kernel(**inputs) takes FULL unsharded inputs (as in reference.setup_inputs())
and returns the full output tuple (tucker_logits [B,N] f32, possibility_score [B,N] f32).

Sharding (8 cores):
  - B (2048) -> 8 x 256 for relation/hr/codebook-gather paths
  - N (20000) -> 8 x 2500 (padded to 2560) for tail features and the [B,N] score matmuls
  - head MLP replicated over full B on every core so BN0 needs no collective
  - ONE AllGather carries the per-core [WmT(bf16); interT(bf16)] shards; BN1 stats
    computed locally from the gathered full-B WmT.

v3 design notes:
  - all matmul inputs bf16; outputs bf16 (host upcasts)
  - MLP biases folded into the PSUM->SBUF evacuation (ACT Relu/Identity with
    per-partition bias AP, or DVE scalar_tensor_tensor) -- no separate bias ops
  - Wm / inter einsums ('bc,bcd->bd'): broadcast tensor_tensor multiply
    (alternating GPSIMD-from-SBUF and DVE-from-PSUM paths) + wide last-axis
    tensor_reduce on DVE with bf16 accumulation
  - ta computed directly in [n, c] layout so topk masking needs no pre-transpose
  - score PSUM evacuated in [128,1024] pair-tiles, round-robin DVE/ACT
"""

import sys

sys.path.insert(0, "/opt/trn_rl_repo")

import numpy as np
import ml_dtypes

import concourse.bass as bass
import concourse.bacc as bacc
import concourse.mybir as mybir
import concourse.tile as tile
from concourse.bass import IndirectOffsetOnAxis
from concourse.bass_utils import run_bass_kernel_spmd
from concourse.masks import make_identity

F32 = mybir.dt.float32
BF16 = mybir.dt.bfloat16
I32 = mybir.dt.int32
AF = mybir.ActivationFunctionType
ALU = mybir.AluOpType
AX = mybir.AxisListType

B, N, E, C, R2 = 2048, 20000, 512, 128, 474
NCORES = 8
BSH = B // NCORES            # 256 b rows per core
NSH = N // NCORES            # 2500 tail rows per core
NPAD = 2560                  # padded to 5 groups of 512
NG = NPAD // 512             # 5 n-groups
NB_FULL = B // 128           # 16 b-tiles over full B
TEMP = 0.5
NEG = -1.0e30

_PROG_CACHE = {}


def build_program():
    nc = bacc.Bacc("TRN2", target_bir_lowering=False, debug=False,
                   num_devices=NCORES)

    # ---------------- DRAM I/O ----------------
    dI = lambda name, shape, dt=BF16: nc.dram_tensor(name, shape, dt, kind="ExternalInput")
    headT = dI("headT", [E, B])                    # full-B head_vector^T (rolled)
    relT = dI("relT", [E, BSH])                    # sharded relation_vector^T
    tailT = dI("tailT", [E, NPAD])                 # sharded+padded tail_vector^T
    relidx = dI("relidx", [BSH, 2], I32)
    cbT = dI("cbT", [2 * R2, C * C // 2])          # tanh-input codebook, rows (d-half, c)
    core_dc = dI("core_dc", [C, C * C])            # core as [e, (d, c)] (c fastest)

    hsw1 = dI("hsw1", [E, E]); rsw1 = dI("rsw1", [E, E])
    tsw1 = dI("tsw1", [E, E]); taw1 = dI("taw1", [E, E])
    hsw2 = dI("hsw2", [E, C]); rsw2 = dI("rsw2", [E, C])
    tsw2 = dI("tsw2", [E, C]); taw2 = dI("taw2", [E, C])
    hrw1 = dI("hrw1", [2 * E, 2 * C])
    hrw2 = dI("hrw2", [2 * C, 2 * C])
    hrw3 = dI("hrw3", [2 * C, C])
    # l1 biases chunked [128, nk] f32; l2 biases [128, 1] f32; tab2 row [1, C]
    hsb1 = dI("hsb1", [128, 4], F32); hsb2 = dI("hsb2", [128, 1], F32)
    rsb1 = dI("rsb1", [128, 4], F32); rsb2 = dI("rsb2", [128, 1], F32)
    tsb1 = dI("tsb1", [128, 4], F32); tsb2 = dI("tsb2", [128, 1], F32)
    tab1 = dI("tab1", [128, 4], F32); tab2 = dI("tab2", [1, C])
    hrb1 = dI("hrb1", [128, 2], F32); hrb2 = dI("hrb2", [128, 2], F32)
    hrb3 = dI("hrb3", [128, 1], F32)
    bn0g = dI("bn0g", [128, 1], F32); bn0b = dI("bn0b", [128, 1], F32)
    bn1g = dI("bn1g", [128, 1], F32); bn1b = dI("bn1b", [128, 1], F32)

    tucker = nc.dram_tensor("tucker", [B, NSH], BF16, kind="ExternalOutput")
    poss = nc.dram_tensor("poss", [B, NSH], BF16, kind="ExternalOutput")

    with tile.TileContext(nc) as tc:
        with (
            tc.tile_pool(name="const", bufs=1) as constp,
            tc.tile_pool(name="w1p", bufs=4) as w1p,         # 4 bufs per family tag
            tc.tile_pool(name="hrw1p", bufs=8) as hrw1p,
            tc.tile_pool(name="w2p", bufs=1) as w2p,
            tc.tile_pool(name="big", bufs=1) as bigp,        # core_dc halves (2 tags)
            tc.tile_pool(name="pcp", bufs=2) as pcp,         # gathered codebook tiles
            tc.tile_pool(name="xt", bufs=8) as xtp,
            tc.tile_pool(name="h1", bufs=2) as h1p,
            tc.tile_pool(name="wsb", bufs=2) as wsbp,        # W 4-chunk sbuf tiles
            tc.tile_pool(name="pers", bufs=1) as pers,
            tc.tile_pool(name="small", bufs=2) as smallp,
            tc.tile_pool(name="stage", bufs=3) as stagep,
            tc.tile_pool(name="psA", bufs=2, space="PSUM") as psA,   # [128,2,512] = 2 banks
            tc.tile_pool(name="psB", bufs=2, space="PSUM") as psB,   # [128,512]   = 1 bank
            tc.tile_pool(name="psC", bufs=1, space="PSUM") as psC,   # [128,128]
            tc.tile_pool(name="dram", bufs=1, space="DRAM") as dramp,
        ):
            # ---------------- constants ----------------
            ident = constp.tile([128, 128], F32)
            make_identity(nc, ident[:])
            ident_bf = constp.tile([128, 128], BF16, tag="idbf")
            nc.gpsimd.tensor_copy(ident_bf[:], ident[:])
            ones_row = constp.tile([1, 512], BF16, tag="ones")
            nc.gpsimd.memset(ones_row[:], 1.0)
            zeros_t = constp.tile([128, 512], BF16, tag="zeros")
            nc.gpsimd.memset(zeros_t[:], 0.0)

            def load_const(dram_t, shape, tag, dt=F32):
                t = constp.tile(shape, dt, tag=tag)
                nc.sync.dma_start(out=t[:], in_=dram_t[:])
                return t

            bias = {}
            for nm, dr, shp, dt in [
                    ("hsb1", hsb1, [128, 4], F32), ("hsb2", hsb2, [128, 1], F32),
                    ("rsb1", rsb1, [128, 4], F32), ("rsb2", rsb2, [128, 1], F32),
                    ("tsb1", tsb1, [128, 4], F32), ("tsb2", tsb2, [128, 1], F32),
                    ("tab1", tab1, [128, 4], F32), ("tab2", tab2, [1, C], BF16),
                    ("hrb1", hrb1, [128, 2], F32), ("hrb2", hrb2, [128, 2], F32),
                    ("hrb3", hrb3, [128, 1], F32)]:
                bias[nm] = load_const(dr, shp, "b_" + nm, dt)
            bn0g_s = load_const(bn0g, [128, 1], "c11")
            bn0b_s = load_const(bn0b, [128, 1], "c12")
            bn1g_s = load_const(bn1g, [128, 1], "c13")
            bn1b_s = load_const(bn1b, [128, 1], "c14")

            def load_w1(w1_dram, nk, tag, pool=None):
                pool = pool or w1p
                w1_t = []
                for k in range(nk):
                    wt = pool.tile([128, w1_dram.shape[1]], BF16, tag=tag)
                    nc.sync.dma_start(out=wt[:], in_=w1_dram[k * 128:(k + 1) * 128, :])
                    w1_t.append(wt)
                return w1_t

            def load_w2(w, nk, tag):
                t = w2p.tile([128, nk, w.shape[1]], BF16, tag=tag)
                nc.sync.dma_start(
                    out=t[:], in_=w[:].rearrange("(k p) c -> p k c", p=128))
                return t

            hsw2_s = load_w2(hsw2, 4, "w2a")
            rsw2_s = load_w2(rsw2, 4, "w2b")
            tsw2_s = load_w2(tsw2, 4, "w2c")
            taw2_s = load_w2(taw2, 4, "w2d")
            hrw2_s = load_w2(hrw2, 2, "w2f")
            hrw3_s = load_w2(hrw3, 2, "w2e")

            HALF = C * C // 2
            core_h = []
            for h in range(2):
                ct = bigp.tile([128, HALF], BF16, tag=f"core{h}", name=f"core{h}")
                nc.sync.dma_start(out=ct[:], in_=core_dc[:, h * HALF:(h + 1) * HALF])
                core_h.append(ct)

            # persistent tiles
            tsT_s = pers.tile([128, NPAD], BF16)      # ts^T (+biases)
            tamT_s = pers.tile([128, NPAD], BF16)     # tam^T
            WmT_all = pers.tile([128, B], BF16)       # gathered Wm^T raw
            intT_all = pers.tile([128, B], BF16)      # gathered inter^T
            WmT_nb = pers.tile([128, B], BF16)        # BN1-applied, score lhsT
            hsT_full = pers.tile([128, B], F32)       # hs^T full B (pre-BN)
            rsT_bf = pers.tile([128, BSH], BF16)
            WmT_sh = pers.tile([128, BSH], BF16)
            intT_sh = pers.tile([128, BSH], BF16)

            # ---------------- engine routing ----------------
            evac_i = [0]

            def evac(out_ap, ps_ap, which=None):
                """PSUM->SBUF copy routed round-robin DVE/ACT."""
                if which is None:
                    evac_i[0] += 1
                    which = "v" if evac_i[0] % 2 else "a"
                if which == "v":
                    nc.vector.tensor_copy(out_ap, ps_ap)
                else:
                    nc.scalar.activation(out_ap, ps_ap, AF.Copy)

            relu_i = [0]

            def relu_bias_evac(out_ap, ps_ap, bias_col):
                """out = relu(ps + bias), [128, nb]; routed DVE/ACT."""
                relu_i[0] += 1
                if relu_i[0] % 2:
                    nb = ps_ap.shape[-1]
                    nc.vector.scalar_tensor_tensor(
                        out_ap, ps_ap, bias_col, zeros_t[:, 0:nb],
                        op0=ALU.add, op1=ALU.max)
                else:
                    nc.scalar.activation(out_ap, ps_ap, AF.Relu, bias=bias_col)

            def bias_evac(out_ap, ps_ap, bias_col):
                """out = ps + bias (per-partition); routed DVE/ACT."""
                evac_i[0] += 1
                if evac_i[0] % 2:
                    nc.vector.tensor_scalar_add(out_ap, ps_ap, bias_col)
                else:
                    nc.scalar.activation(out_ap, ps_ap, AF.Identity, bias=bias_col)

            def mm(out, lhsT, rhs, start=True, stop=True):
                nc.tensor.matmul(out, lhsT, rhs, start=start, stop=stop)

            # ---------------- MLP helpers ----------------
            def load_xt(xT_dram, col0, nb, nk):
                xt_t = []
                for k in range(nk):
                    xt = xtp.tile([128, nb], BF16, tag=f"xt{nb}")
                    nc.sync.dma_start(
                        out=xt[:], in_=xT_dram[k * 128:(k + 1) * 128, col0:col0 + nb])
                    xt_t.append(xt)
                return xt_t

            def mlp_l1(w1_t, b1c, xt_t, nb, nm=4):
                """h1 [128, nm, nb] bf16 = relu(x @ w1 + b1)^T in m-chunks."""
                h1 = h1p.tile([128, nm, nb], BF16, tag=f"h1_{nm}_{nb}")
                for half in range(nm // 2):
                    ps = psA.tile([128, 2, nb], F32, tag="A")
                    for mi in range(2):
                        m = half * 2 + mi
                        for k in range(len(w1_t)):
                            mm(ps[:, mi, :], w1_t[k][:, m * 128:(m + 1) * 128],
                               xt_t[k][:], start=(k == 0), stop=(k == len(w1_t) - 1))
                        relu_bias_evac(h1[:, half * 2 + mi, :], ps[:, mi, :],
                                       b1c[:, m:m + 1])
                return h1

            def mlp_l2_T(w2_s, b2c, h1, nb, out_ap, nm=4):
                """out [128 c, nb] = (h1 @ w2 + b2)^T."""
                ps2 = psB.tile([128, nb], F32, tag="B")
                for m in range(nm):
                    mm(ps2[:], w2_s[:, m, :], h1[:, m, :], start=(m == 0),
                       stop=(m == nm - 1))
                bias_evac(out_ap, ps2[:], b2c[:, 0:1])

            # ---------- soft top-10 mask ([128,128] f32 AP, may be PSUM) ----------
            def topk_mask_mul(x_ap, out_ap):
                """out = sigmoid((x - thr10)/TEMP) * x   (out bf16)"""
                m8 = smallp.tile([128, 8], F32, tag="m8")
                zap = smallp.tile([128, 128], F32, tag="zap")
                nc.vector.max(out=m8[:], in_=x_ap)
                nc.vector.match_replace(out=zap[:], in_to_replace=m8[:],
                                        in_values=x_ap, imm_value=NEG)
                nc.vector.max(out=m8[:], in_=zap[:])
                thr = smallp.tile([128, 1], F32, tag="thr")
                nc.vector.tensor_scalar_mul(thr[:], m8[:, 1:2], -1.0 / TEMP)
                mask = smallp.tile([128, 128], F32, tag="mask")
                nc.scalar.activation(mask[:], x_ap, AF.Sigmoid,
                                     bias=thr[:, 0:1], scale=1.0 / TEMP)
                nc.vector.tensor_mul(out_ap, mask[:], x_ap)

            # =========== Phase 1: head MLP (full B) ===========
            hsw1_t = load_w1(hsw1, 4, "w1hs")
            for bg in range(B // 512):
                xt_bg = load_xt(headT, bg * 512, 512, 4)
                h1h = mlp_l1(hsw1_t, bias["hsb1"], xt_bg, 512)
                mlp_l2_T(hsw2_s, bias["hsb2"], h1h, 512,
                         hsT_full[:, bg * 512:(bg + 1) * 512])

            # =========== rel MLP ===========
            xt_r = load_xt(relT, 0, BSH, 4)
            rsw1_t = load_w1(rsw1, 4, "w1rs")
            h1r = mlp_l1(rsw1_t, bias["rsb1"], xt_r, BSH)
            mlp_l2_T(rsw2_s, bias["rsb2"], h1r, BSH, rsT_bf[:])

            # =========== hr MLP (3 layers) ===========
            hrw1_t = load_w1(hrw1, 8, "w1hr", hrw1p)
            hr_x = load_xt(headT, 0, BSH, 4) + xt_r   # concat(head_my, rel)
            hrh1 = h1p.tile([128, 2, BSH], BF16, tag="hrh1")
            ps = psA.tile([128, 2, BSH], F32, tag="A")
            for mi in range(2):
                for k in range(8):
                    mm(ps[:, mi, :], hrw1_t[k][:, mi * 128:(mi + 1) * 128],
                       hr_x[k][:], start=(k == 0), stop=(k == 7))
                relu_bias_evac(hrh1[:, mi, :], ps[:, mi, :], bias["hrb1"][:, mi:mi + 1])
            hrh2 = h1p.tile([128, 2, BSH], BF16, tag="hrh2")
            ps = psA.tile([128, 2, BSH], F32, tag="A")
            for mi in range(2):
                for k in range(2):
                    mm(ps[:, mi, :], hrw2_s[:, k, mi * 128:(mi + 1) * 128],
                       hrh1[:, k, :], start=(k == 0), stop=(k == 1))
                relu_bias_evac(hrh2[:, mi, :], ps[:, mi, :], bias["hrb2"][:, mi:mi + 1])
            hraT = smallp.tile([128, BSH], F32, tag="hraT")
            ps2 = psB.tile([128, BSH], F32, tag="B")
            for k in range(2):
                mm(ps2[:], hrw3_s[:, k, :], hrh2[:, k, :], start=(k == 0),
                   stop=(k == 1))
            bias_evac(hraT[:], ps2[:], bias["hrb3"][:, 0:1])

            # hrm (masked hra) in [b, c] layout, bf16, per t-tile
            hrm_bf = []
            for t in range(2):
                pst = psC.tile([128, 128], F32, tag="C")
                nc.tensor.transpose(pst[:], hraT[:, t * 128:(t + 1) * 128], ident[:])
                hb = smallp.tile([128, 128], BF16, tag="hrmbf")
                topk_mask_mul(pst[:], hb[:])
                hrm_bf.append(hb)

            # =========== BN0 (local, full B) ===========
            def bn_finalize(stats_ap, g_tile, b_tile):
                mv = smallp.tile([128, 2], F32, tag="mv")
                nc.vector.bn_aggr(mv[:], stats_ap)
                scale = smallp.tile([128, 1], F32, tag="sc")
                shift = smallp.tile([128, 1], F32, tag="sh")
                tmp = smallp.tile([128, 1], F32, tag="tm")
                nc.vector.tensor_scalar_add(tmp[:], mv[:, 1:2], 1e-5)
                nc.scalar.activation(scale[:], tmp[:], AF.Sqrt)
                nc.vector.reciprocal(scale[:], scale[:])
                nc.vector.tensor_mul(scale[:], scale[:], g_tile[:, 0:1])
                nc.vector.tensor_mul(tmp[:], mv[:, 0:1], scale[:])
                nc.vector.tensor_sub(shift[:], b_tile[:, 0:1], tmp[:])
                return scale, shift

            st0 = smallp.tile([128, 4, 6], F32, tag="st0")
            for i in range(4):
                nc.vector.bn_stats(st0[:, i, :], hsT_full[:, i * 512:(i + 1) * 512])
            bn0_scale, bn0_shift = bn_finalize(st0[:], bn0g_s, bn0b_s)

            # ha (bn0-applied hs shard) in [b, c] bf16 tiles
            haT_bf = smallp.tile([128, BSH], BF16, tag="haTbf")
            nc.vector.tensor_scalar(haT_bf[:], hsT_full[:, 0:BSH], bn0_scale[:, 0:1],
                                    bn0_shift[:, 0:1], op0=ALU.mult, op1=ALU.add)
            ha_bf = []
            for t in range(2):
                pst = psC.tile([128, 128], BF16, tag="Cb")
                nc.tensor.transpose(pst[:], haT_bf[:, t * 128:(t + 1) * 128],
                                    ident_bf[:])
                hb = smallp.tile([128, 128], BF16, tag="habf")
                evac(hb[:], pst[:])
                ha_bf.append(hb)

            # Wm / inter shard accumulators in [b, d] layout
            Wm_sh = [smallp.tile([128, 128], BF16, tag=f"wmsh{t}",
                                 name=f"Wm_sh{t}") for t in range(2)]
            inter_sh = [smallp.tile([128, 128], BF16, tag=f"ish{t}",
                                    name=f"inter_sh{t}") for t in range(2)]
            idx_t = []
            for t in range(2):
                idx = smallp.tile([128, 2], I32, tag=f"idx{t}")
                nc.sync.dma_start(out=idx[:], in_=relidx[t * 128:(t + 1) * 128, :])
                idx_t.append(idx)

            # =========== tail weights ===========
            tsw1_t = load_w1(tsw1, 4, "w1ts")
            taw1_t = load_w1(taw1, 4, "w1ta")

            # ---------------- W 4-chunk group (einsum via GPS or DVE) ----------------
            def w_group(gi):
                t, q = divmod(gi, 8)          # q = which 4-chunk quarter (d0 = 16q)
                use_gps = (gi % 2 == 0)
                wsb = wsbp.tile([128, 4, 4, 128], BF16, tag="wsb")
                for jj in range(4):
                    j = q * 4 + jj            # chunk index 0..31 (d = 4j..4j+3)
                    hsel, off = divmod(j * 512, HALF)
                    pw = psB.tile([128, 512], F32, tag="B")
                    mm(pw[:], rsT_bf[:, t * 128:(t + 1) * 128],
                       core_h[hsel][:, off:off + 512])
                    if use_gps:
                        evac(wsb[:, jj, :, :],
                             pw[:].rearrange("p (d c) -> p d c", c=128))
                    else:
                        # DVE multiply straight from PSUM (1x), write bf16 SBUF
                        nc.vector.tensor_tensor(
                            out=wsb[:, jj, :, :],
                            in0=pw[:].rearrange("p (d c) -> p d c", c=128),
                            in1=ha_bf[t][:, None, :].to_broadcast([128, 4, 128]),
                            op=ALU.mult)
                w4 = wsb[:].rearrange("p j d c -> p (j d) c")
                if use_gps:
                    nc.gpsimd.tensor_tensor(
                        out=w4, in0=w4,
                        in1=ha_bf[t][:, None, :].to_broadcast([128, 16, 128]),
                        op=ALU.mult)
                with nc.allow_low_precision("bf16 Wm accumulate"):
                    nc.vector.tensor_reduce(Wm_sh[t][:, 16 * q:16 * (q + 1)],
                                            w4, axis=AX.X, op=ALU.add)

            # ---------------- inter tile (t, h) ----------------
            def inter_tile(t, h):
                pc = pcp.tile([128, HALF], BF16, tag="pc")
                nc.gpsimd.indirect_dma_start(
                    out=pc[:], out_offset=None, in_=cbT[:],
                    in_offset=IndirectOffsetOnAxis(ap=idx_t[t][:, h:h + 1], axis=0))
                nc.scalar.activation(pc[:], pc[:], AF.Tanh)
                pc3 = pc[:].rearrange("p (d c) -> p d c", c=128)
                nc.gpsimd.tensor_tensor(
                    out=pc3, in0=pc3,
                    in1=hrm_bf[t][:, None, :].to_broadcast([128, 64, 128]),
                    op=ALU.mult)
                with nc.allow_low_precision("bf16 inter accumulate"):
                    nc.vector.tensor_reduce(inter_sh[t][:, h * 64:(h + 1) * 64],
                                            pc3, axis=AX.X, op=ALU.add)

            # ---------------- tail MLP group ----------------
            def tail_group(g):
                xt_g = load_xt(tailT, g * 512, 512, 4)
                h1t = mlp_l1(tsw1_t, bias["tsb1"], xt_g, 512)
                mlp_l2_T(tsw2_s, bias["tsb2"], h1t, 512,
                         tsT_s[:, g * 512:(g + 1) * 512])
                h1a = mlp_l1(taw1_t, bias["tab1"], xt_g, 512)
                for nt in range(4):
                    pt = psC.tile([128, 128], F32, tag="C")
                    for m in range(4):
                        mm(pt[:], h1a[:, m, nt * 128:(nt + 1) * 128],
                           taw2_s[:, m, :], start=(m == 0), stop=False)
                    mm(pt[:], ones_row[0:1, 0:128], bias["tab2"][0:1, :],
                       start=False, stop=True)
                    tam_nc = smallp.tile([128, 128], BF16, tag="tamnc")
                    topk_mask_mul(pt[:], tam_nc[:])
                    ptT = psC.tile([128, 128], BF16, tag="Cb")
                    nc.tensor.transpose(ptT[:], tam_nc[:], ident_bf[:])
                    evac(tamT_s[:, g * 512 + nt * 128:g * 512 + (nt + 1) * 128],
                         ptT[:])

            # =========== Phase 2: interleaved tail / Wm / inter ===========
            WG_SCHED = [3, 3, 3, 3, 4]      # w_groups emitted after each tail group
            wg_next = 0
            for g in range(NG):
                tail_group(g)
                for _ in range(WG_SCHED[g]):
                    w_group(wg_next)
                    wg_next += 1
                if g < 4:
                    inter_tile(g // 2, g % 2)

            # =========== Phase 3: pack shard, AllGather, BN1 ===========
            for t in range(2):
                pst = psC.tile([128, 128], BF16, tag="Cb")
                nc.tensor.transpose(pst[:], Wm_sh[t][:], ident_bf[:])
                evac(WmT_sh[:, t * 128:(t + 1) * 128], pst[:])
                pst2 = psC.tile([128, 128], BF16, tag="Cb")
                nc.tensor.transpose(pst2[:], inter_sh[t][:], ident_bf[:])
                evac(intT_sh[:, t * 128:(t + 1) * 128], pst2[:])

            ag2_in = dramp.tile([2, 128, BSH], BF16)
            ag2_out = dramp.tile([NCORES, 2, 128, BSH], BF16, addr_space="Shared")
            nc.sync.dma_start(out=ag2_in[0], in_=WmT_sh[:])
            nc.sync.dma_start(out=ag2_in[1], in_=intT_sh[:])
            nc.gpsimd.collective_compute(
                "AllGather", ALU.bypass,
                replica_groups=[list(range(NCORES))],
                ins=[ag2_in.opt()], outs=[ag2_out.opt()])
            nc.sync.dma_start(
                out=WmT_all[:], in_=ag2_out[:, 0].rearrange("r d b -> d r b"))
            nc.sync.dma_start(
                out=intT_all[:], in_=ag2_out[:, 1].rearrange("r d b -> d r b"))

            # BN1 on gathered WmT (full B)
            st1 = smallp.tile([128, 4, 6], F32, tag="st1")
            for i in range(4):
                nc.vector.bn_stats(st1[:, i, :], WmT_all[:, i * 512:(i + 1) * 512])
            bn1_scale, bn1_shift = bn_finalize(st1[:], bn1g_s, bn1b_s)
            nc.vector.tensor_scalar(WmT_nb[:], WmT_all[:], bn1_scale[:, 0:1],
                                    bn1_shift[:, 0:1], op0=ALU.mult, op1=ALU.add)

            # =========== Phase 4: scores ===========
            NLAST = NSH - 4 * 512               # 452 valid cols in group 4
            for bt in range(NB_FULL):
                r0 = bt * 128
                for lhsT, outd, rhs in ((WmT_nb, tucker, tsT_s),
                                        (intT_all, poss, tamT_s)):
                    for half in range(2):
                        psq = psA.tile([128, 2, 512], F32, tag="A")
                        for gg in range(2):
                            g = half * 2 + gg
                            mm(psq[:, gg, :], lhsT[:, r0:r0 + 128],
                               rhs[:, g * 512:(g + 1) * 512])
                        stq = stagep.tile([128, 1024], BF16, tag="stq")
                        evac(stq[:], psq[:].rearrange("p g n -> p (g n)"))
                        nc.sync.dma_start(
                            out=outd[r0:r0 + 128, half * 1024:(half + 1) * 1024],
                            in_=stq[:])
                    ps4 = psB.tile([128, 512], F32, tag="B")
                    mm(ps4[:], lhsT[:, r0:r0 + 128], rhs[:, 4 * 512:5 * 512])
                    st4 = stagep.tile([128, 512], BF16, tag="st4")
                    evac(st4[:], ps4[:])
                    nc.sync.dma_start(out=outd[r0:r0 + 128, 2048:NSH],
                                      in_=st4[:, 0:NLAST])

    nc.finalize()
    return nc


# ---------------------------------------------------------------------------
# host side
# ---------------------------------------------------------------------------

BF = np.dtype(ml_dtypes.bfloat16)


def _to_np(x, dt=np.float32):
    return np.ascontiguousarray(np.asarray(x), dtype=dt)


def prepare_in_maps(inputs):
    head = _to_np(inputs["head_vector"])        # [B, E]
    rel = _to_np(inputs["relation_vector"])     # [B, E]
    ridx = np.ascontiguousarray(np.asarray(inputs["relation_index"]).astype(np.int32))
    tailv = _to_np(inputs["tail_vector"])       # [N, E]
    codebook = _to_np(inputs["codebook"])       # [R2, C, C]
    core = _to_np(inputs["core"])               # [C, C, C]

    cbT_host = np.ascontiguousarray(
        codebook.transpose(0, 2, 1).reshape(2 * R2, C * C // 2)).astype(BF)
    core_dc_host = np.ascontiguousarray(
        core.transpose(0, 2, 1).reshape(C, C * C)).astype(BF)

    headT_full = np.ascontiguousarray(head.T).astype(BF)   # [E, B]
    relT_full = np.ascontiguousarray(rel.T).astype(BF)     # [E, B]
    tailT_full = np.ascontiguousarray(tailv.T).astype(BF)  # [E, N]

    def chunked_bias(k, nk):
        return np.ascontiguousarray(_to_np(inputs[k]).reshape(nk, 128).T)

    col = lambda k: _to_np(inputs[k]).reshape(128, 1)
    wcast = lambda k: _to_np(inputs[k]).astype(BF)
    weights_common = {
        "hsw1": wcast("hsw1"), "hsb1": chunked_bias("hsb1", 4),
        "hsw2": wcast("hsw2"), "hsb2": col("hsb2"),
        "rsw1": wcast("rsw1"), "rsb1": chunked_bias("rsb1", 4),
        "rsw2": wcast("rsw2"), "rsb2": col("rsb2"),
        "tsw1": wcast("tsw1"), "tsb1": chunked_bias("tsb1", 4),
        "tsw2": wcast("tsw2"), "tsb2": col("tsb2"),
        "taw1": wcast("taw1"), "tab1": chunked_bias("tab1", 4),
        "taw2": wcast("taw2"),
        "tab2": _to_np(inputs["tab2"]).reshape(1, C).astype(BF),
        "hrw1": wcast("hrw1"), "hrb1": chunked_bias("hrb1", 2),
        "hrw2": wcast("hrw2"), "hrb2": chunked_bias("hrb2", 2),
        "hrw3": wcast("hrw3"), "hrb3": col("hrb3"),
        "bn0g": col("bn0_g"), "bn0b": col("bn0_b"),
        "bn1g": col("bn1_g"), "bn1b": col("bn1_b"),
        "cbT": cbT_host, "core_dc": core_dc_host,
    }

    in_maps = []
    for k in range(NCORES):
        b0 = k * BSH
        n0 = k * NSH
        # rotate headT so THIS core's 256 b-columns come first; BN0 stats are
        # order-invariant and slices [0:256] are "my" shard on every core.
        headT_k = np.ascontiguousarray(np.roll(headT_full, -b0, axis=1))
        tailT_k = np.zeros((E, NPAD), BF)
        tailT_k[:, :NSH] = tailT_full[:, n0:n0 + NSH]
        m = dict(weights_common)
        m["headT"] = headT_k
        m["relT"] = np.ascontiguousarray(relT_full[:, b0:b0 + BSH])
        m["tailT"] = tailT_k
        ri = ridx[b0:b0 + BSH]
        m["relidx"] = np.ascontiguousarray(
            np.stack([2 * ri, 2 * ri + 1], axis=1))
        in_maps.append(m)
    return in_maps


def assemble_outputs(results):
    tuckers, posses = [], []
    for k in range(NCORES):
        r = results[k]
        tuckers.append(np.asarray(r["tucker"]).astype(np.float32))
        posses.append(np.asarray(r["poss"]).astype(np.float32))
    tucker_full = np.concatenate(tuckers, axis=1)
    poss_full = np.concatenate(posses, axis=1)
    return tucker_full, poss_full


def kernel(**inputs):
    if "prog" not in _PROG_CACHE:
        _PROG_CACHE["prog"] = build_program()
    nc = _PROG_CACHE["prog"]
    in_maps = prepare_in_maps(inputs)
    res = run_bass_kernel_spmd(nc, in_maps, list(range(NCORES)))
    return assemble_outputs(res.results)


# revision 17
# speedup vs baseline: 1.3336x; 1.0299x over previous
"""Trainium2 Bass kernel for nn_BaseModel_74302934220896 (TuckER + possibility-codebook).

Contract: kernel(**inputs) takes FULL unsharded inputs (as in reference.setup_inputs())
and returns the full output tuple (tucker_logits [B,N] f32, possibility_score [B,N] f32).

Sharding (8 cores):
  - B (2048) -> 8 x 256 for relation/hr/codebook-gather paths
  - N (20000) -> 8 x 2500 (padded to 2560) for tail features and the [B,N] score matmuls
  - head MLP replicated over full B on every core so BN0 needs no collective
  - ONE AllGather carries the per-core [WmT(bf16); interT(bf16)] shards; BN1 stats
    computed locally from the gathered full-B WmT.

v3 design notes:
  - all matmul inputs bf16; outputs bf16 (host upcasts)
  - MLP biases folded into the PSUM->SBUF evacuation (ACT Relu/Identity with
    per-partition bias AP, or DVE scalar_tensor_tensor) -- no separate bias ops
  - Wm / inter einsums ('bc,bcd->bd'): broadcast tensor_tensor multiply
    (alternating GPSIMD-from-SBUF and DVE-from-PSUM paths) + wide last-axis
    tensor_reduce on DVE with bf16 accumulation
  - ta computed directly in [n, c] layout so topk masking needs no pre-transpose
  - score PSUM evacuated in [128,1024] pair-tiles, round-robin DVE/ACT
"""

import sys

sys.path.insert(0, "/opt/trn_rl_repo")

import numpy as np
import ml_dtypes

import concourse.bass as bass
import concourse.bacc as bacc
import concourse.mybir as mybir
import concourse.tile as tile
from concourse.bass import IndirectOffsetOnAxis
from concourse.bass_utils import run_bass_kernel_spmd
from concourse.masks import make_identity

F32 = mybir.dt.float32
BF16 = mybir.dt.bfloat16
I32 = mybir.dt.int32
AF = mybir.ActivationFunctionType
ALU = mybir.AluOpType
AX = mybir.AxisListType

B, N, E, C, R2 = 2048, 20000, 512, 128, 474
NCORES = 8
BSH = B // NCORES            # 256 b rows per core
NSH = N // NCORES            # 2500 tail rows per core
NPAD = 2560                  # padded to 5 groups of 512
NG = NPAD // 512             # 5 n-groups
NB_FULL = B // 128           # 16 b-tiles over full B
TEMP = 0.5
NEG = -1.0e30

_PROG_CACHE = {}


def build_program():
    nc = bacc.Bacc("TRN2", target_bir_lowering=False, debug=False,
                   num_devices=NCORES)

    # ---------------- DRAM I/O ----------------
    dI = lambda name, shape, dt=BF16: nc.dram_tensor(name, shape, dt, kind="ExternalInput")
    headT = dI("headT", [E, B])                    # full-B head_vector^T (rolled)
    relT = dI("relT", [E, BSH])                    # sharded relation_vector^T
    tailT = dI("tailT", [E, NPAD])                 # sharded+padded tail_vector^T
    relidx = dI("relidx", [BSH, 2], I32)
    cbT = dI("cbT", [2 * R2, C * C // 2])          # tanh-input codebook, rows (d-half, c)
    core_dc = dI("core_dc", [C, C * C])            # core as [e, (d, c)] (c fastest)

    hsw1 = dI("hsw1", [E, E]); rsw1 = dI("rsw1", [E, E])
    tsw1 = dI("tsw1", [E, E]); taw1 = dI("taw1", [E, E])
    hsw2 = dI("hsw2", [E, C]); rsw2 = dI("rsw2", [E, C])
    tsw2 = dI("tsw2", [E, C]); taw2 = dI("taw2", [E, C])
    hrw1 = dI("hrw1", [2 * E, 2 * C])
    hrw2 = dI("hrw2", [2 * C, 2 * C])
    hrw3 = dI("hrw3", [2 * C, C])
    # l1 biases chunked [128, nk] f32; l2 biases [128, 1] f32; tab2 row [1, C]
    hsb1 = dI("hsb1", [128, 4], F32); hsb2 = dI("hsb2", [128, 1], F32)
    rsb1 = dI("rsb1", [128, 4], F32); rsb2 = dI("rsb2", [128, 1], F32)
    tsb1 = dI("tsb1", [128, 4], F32); tsb2 = dI("tsb2", [128, 1], F32)
    tab1 = dI("tab1", [128, 4], F32); tab2 = dI("tab2", [1, C])
    hrb1 = dI("hrb1", [128, 2], F32); hrb2 = dI("hrb2", [128, 2], F32)
    hrb3 = dI("hrb3", [128, 1], F32)
    bn0g = dI("bn0g", [128, 1], F32); bn0b = dI("bn0b", [128, 1], F32)
    bn1g = dI("bn1g", [128, 1], F32); bn1b = dI("bn1b", [128, 1], F32)

    tucker = nc.dram_tensor("tucker", [B, NSH], BF16, kind="ExternalOutput")
    poss = nc.dram_tensor("poss", [B, NSH], BF16, kind="ExternalOutput")

    with tile.TileContext(nc) as tc:
        with (
            tc.tile_pool(name="const", bufs=1) as constp,
            tc.tile_pool(name="w1p", bufs=4) as w1p,         # 4 bufs per family tag
            tc.tile_pool(name="hrw1p", bufs=8) as hrw1p,
            tc.tile_pool(name="w2p", bufs=1) as w2p,
            tc.tile_pool(name="big", bufs=1) as bigp,        # core_dc halves (2 tags)
            tc.tile_pool(name="pcp", bufs=2) as pcp,         # gathered codebook tiles
            tc.tile_pool(name="xt", bufs=8) as xtp,
            tc.tile_pool(name="h1", bufs=2) as h1p,
            tc.tile_pool(name="wsb", bufs=2) as wsbp,        # W 4-chunk sbuf tiles
            tc.tile_pool(name="pers", bufs=1) as pers,
            tc.tile_pool(name="small", bufs=2) as smallp,
            tc.tile_pool(name="stage", bufs=3) as stagep,
            tc.tile_pool(name="psA", bufs=2, space="PSUM") as psA,   # [128,2,512] = 2 banks
            tc.tile_pool(name="psB", bufs=2, space="PSUM") as psB,   # [128,512]   = 1 bank
            tc.tile_pool(name="dram", bufs=1, space="DRAM") as dramp,
        ):
            # psC lives through phases 1-3, then is released so psD (scores
            # pipeline depth-3) can use its banks in phase 4.
            psC = tc.alloc_tile_pool(name="psC", bufs=1, space="PSUM")
            # ---------------- constants ----------------
            ident = constp.tile([128, 128], F32)
            make_identity(nc, ident[:])
            ident_bf = constp.tile([128, 128], BF16, tag="idbf")
            nc.gpsimd.tensor_copy(ident_bf[:], ident[:])
            ones_row = constp.tile([1, 512], BF16, tag="ones")
            nc.gpsimd.memset(ones_row[:], 1.0)
            zeros_t = constp.tile([128, 512], BF16, tag="zeros")
            nc.gpsimd.memset(zeros_t[:], 0.0)

            def load_const(dram_t, shape, tag, dt=F32):
                t = constp.tile(shape, dt, tag=tag)
                nc.sync.dma_start(out=t[:], in_=dram_t[:])
                return t

            bias = {}
            for nm, dr, shp, dt in [
                    ("hsb1", hsb1, [128, 4], F32), ("hsb2", hsb2, [128, 1], F32),
                    ("rsb1", rsb1, [128, 4], F32), ("rsb2", rsb2, [128, 1], F32),
                    ("tsb1", tsb1, [128, 4], F32), ("tsb2", tsb2, [128, 1], F32),
                    ("tab1", tab1, [128, 4], F32), ("tab2", tab2, [1, C], BF16),
                    ("hrb1", hrb1, [128, 2], F32), ("hrb2", hrb2, [128, 2], F32),
                    ("hrb3", hrb3, [128, 1], F32)]:
                bias[nm] = load_const(dr, shp, "b_" + nm, dt)
            bn0g_s = load_const(bn0g, [128, 1], "c11")
            bn0b_s = load_const(bn0b, [128, 1], "c12")
            bn1g_s = load_const(bn1g, [128, 1], "c13")
            bn1b_s = load_const(bn1b, [128, 1], "c14")

            def load_w1(w1_dram, nk, tag, pool=None):
                pool = pool or w1p
                w1_t = []
                for k in range(nk):
                    wt = pool.tile([128, w1_dram.shape[1]], BF16, tag=tag)
                    nc.sync.dma_start(out=wt[:], in_=w1_dram[k * 128:(k + 1) * 128, :])
                    w1_t.append(wt)
                return w1_t

            def load_w2(w, nk, tag):
                t = w2p.tile([128, nk, w.shape[1]], BF16, tag=tag)
                nc.sync.dma_start(
                    out=t[:], in_=w[:].rearrange("(k p) c -> p k c", p=128))
                return t

            hsw2_s = load_w2(hsw2, 4, "w2a")
            rsw2_s = load_w2(rsw2, 4, "w2b")
            tsw2_s = load_w2(tsw2, 4, "w2c")
            taw2_s = load_w2(taw2, 4, "w2d")
            hrw2_s = load_w2(hrw2, 2, "w2f")
            hrw3_s = load_w2(hrw3, 2, "w2e")

            HALF = C * C // 2
            core_h = []
            for h in range(2):
                ct = bigp.tile([128, HALF], BF16, tag=f"core{h}", name=f"core{h}")
                nc.sync.dma_start(out=ct[:], in_=core_dc[:, h * HALF:(h + 1) * HALF])
                core_h.append(ct)

            # persistent tiles
            tsT_s = pers.tile([128, NPAD], BF16)      # ts^T (+biases)
            tamT_s = pers.tile([128, NPAD], BF16)     # tam^T
            WmT_all = pers.tile([128, B], BF16)       # gathered Wm^T raw
            intT_all = pers.tile([128, B], BF16)      # gathered inter^T
            WmT_nb = pers.tile([128, B], BF16)        # BN1-applied, score lhsT
            hsT_full = pers.tile([128, B], F32)       # hs^T full B (pre-BN)
            rsT_bf = pers.tile([128, BSH], BF16)
            WmT_sh = pers.tile([128, BSH], BF16)
            intT_sh = pers.tile([128, BSH], BF16)

            # ---------------- engine routing ----------------
            evac_i = [0]

            def evac(out_ap, ps_ap, which=None):
                """PSUM->SBUF copy routed round-robin DVE/ACT."""
                if which is None:
                    evac_i[0] += 1
                    which = "v" if evac_i[0] % 2 else "a"
                if which == "v":
                    nc.vector.tensor_copy(out_ap, ps_ap)
                else:
                    nc.scalar.activation(out_ap, ps_ap, AF.Copy)

            relu_i = [0]

            def relu_bias_evac(out_ap, ps_ap, bias_col):
                """out = relu(ps + bias), [128, nb]; ACT-biased routing (2a:1v)."""
                relu_i[0] += 1
                if relu_i[0] % 3 == 0:
                    nb = ps_ap.shape[-1]
                    nc.vector.scalar_tensor_tensor(
                        out_ap, ps_ap, bias_col, zeros_t[:, 0:nb],
                        op0=ALU.add, op1=ALU.max)
                else:
                    nc.scalar.activation(out_ap, ps_ap, AF.Relu, bias=bias_col)

            def bias_evac(out_ap, ps_ap, bias_col):
                """out = ps + bias (per-partition); ACT-biased routing."""
                evac_i[0] += 1
                if evac_i[0] % 3 == 0:
                    nc.vector.tensor_scalar_add(out_ap, ps_ap, bias_col)
                else:
                    nc.scalar.activation(out_ap, ps_ap, AF.Identity, bias=bias_col)

            def mm(out, lhsT, rhs, start=True, stop=True):
                nc.tensor.matmul(out, lhsT, rhs, start=start, stop=stop)

            # ---------------- MLP helpers ----------------
            def load_xt(xT_dram, col0, nb, nk):
                xt_t = []
                for k in range(nk):
                    xt = xtp.tile([128, nb], BF16, tag=f"xt{nb}")
                    nc.sync.dma_start(
                        out=xt[:], in_=xT_dram[k * 128:(k + 1) * 128, col0:col0 + nb])
                    xt_t.append(xt)
                return xt_t

            def mlp_l1(w1_t, b1c, xt_t, nb, nm=4):
                """h1 [128, nm, nb] bf16 = relu(x @ w1 + b1)^T in m-chunks."""
                h1 = h1p.tile([128, nm, nb], BF16, tag=f"h1_{nm}_{nb}")
                for half in range(nm // 2):
                    ps = psA.tile([128, 2, nb], F32, tag="A")
                    for mi in range(2):
                        m = half * 2 + mi
                        for k in range(len(w1_t)):
                            mm(ps[:, mi, :], w1_t[k][:, m * 128:(m + 1) * 128],
                               xt_t[k][:], start=(k == 0), stop=(k == len(w1_t) - 1))
                        relu_bias_evac(h1[:, half * 2 + mi, :], ps[:, mi, :],
                                       b1c[:, m:m + 1])
                return h1

            def mlp_l2_T(w2_s, b2c, h1, nb, out_ap, nm=4):
                """out [128 c, nb] = (h1 @ w2 + b2)^T."""
                ps2 = psB.tile([128, nb], F32, tag="B")
                for m in range(nm):
                    mm(ps2[:], w2_s[:, m, :], h1[:, m, :], start=(m == 0),
                       stop=(m == nm - 1))
                bias_evac(out_ap, ps2[:], b2c[:, 0:1])

            # ---------- soft top-10 mask ([128,128] f32 AP, may be PSUM) ----------
            def topk_mask_mul(x_ap, out_ap):
                """out = sigmoid((x - thr10)/TEMP) * x   (out bf16)"""
                m8 = smallp.tile([128, 8], F32, tag="m8")
                zap = smallp.tile([128, 128], F32, tag="zap")
                nc.vector.max(out=m8[:], in_=x_ap)
                nc.vector.match_replace(out=zap[:], in_to_replace=m8[:],
                                        in_values=x_ap, imm_value=NEG)
                nc.vector.max(out=m8[:], in_=zap[:])
                thr = smallp.tile([128, 1], F32, tag="thr")
                nc.vector.tensor_scalar_mul(thr[:], m8[:, 1:2], -1.0 / TEMP)
                mask = smallp.tile([128, 128], F32, tag="mask")
                nc.scalar.activation(mask[:], x_ap, AF.Sigmoid,
                                     bias=thr[:, 0:1], scale=1.0 / TEMP)
                nc.vector.tensor_mul(out_ap, mask[:], x_ap)

            # =========== Phase 1: head MLP (full B) ===========
            hsw1_t = load_w1(hsw1, 4, "w1hs")
            for bg in range(B // 512):
                xt_bg = load_xt(headT, bg * 512, 512, 4)
                h1h = mlp_l1(hsw1_t, bias["hsb1"], xt_bg, 512)
                mlp_l2_T(hsw2_s, bias["hsb2"], h1h, 512,
                         hsT_full[:, bg * 512:(bg + 1) * 512])

            # =========== rel MLP ===========
            xt_r = load_xt(relT, 0, BSH, 4)
            rsw1_t = load_w1(rsw1, 4, "w1rs")
            h1r = mlp_l1(rsw1_t, bias["rsb1"], xt_r, BSH)
            mlp_l2_T(rsw2_s, bias["rsb2"], h1r, BSH, rsT_bf[:])

            # =========== hr MLP (3 layers) ===========
            hrw1_t = load_w1(hrw1, 8, "w1hr", hrw1p)
            hr_x = load_xt(headT, 0, BSH, 4) + xt_r   # concat(head_my, rel)
            hrh1 = h1p.tile([128, 2, BSH], BF16, tag="hrh1")
            ps = psA.tile([128, 2, BSH], F32, tag="A")
            for mi in range(2):
                for k in range(8):
                    mm(ps[:, mi, :], hrw1_t[k][:, mi * 128:(mi + 1) * 128],
                       hr_x[k][:], start=(k == 0), stop=(k == 7))
                relu_bias_evac(hrh1[:, mi, :], ps[:, mi, :], bias["hrb1"][:, mi:mi + 1])
            hrh2 = h1p.tile([128, 2, BSH], BF16, tag="hrh2")
            ps = psA.tile([128, 2, BSH], F32, tag="A")
            for mi in range(2):
                for k in range(2):
                    mm(ps[:, mi, :], hrw2_s[:, k, mi * 128:(mi + 1) * 128],
                       hrh1[:, k, :], start=(k == 0), stop=(k == 1))
                relu_bias_evac(hrh2[:, mi, :], ps[:, mi, :], bias["hrb2"][:, mi:mi + 1])
            hraT = smallp.tile([128, BSH], F32, tag="hraT")
            ps2 = psB.tile([128, BSH], F32, tag="B")
            for k in range(2):
                mm(ps2[:], hrw3_s[:, k, :], hrh2[:, k, :], start=(k == 0),
                   stop=(k == 1))
            bias_evac(hraT[:], ps2[:], bias["hrb3"][:, 0:1])

            # hrm (masked hra) in [b, c] layout, bf16, per t-tile
            hrm_bf = []
            for t in range(2):
                pst = psC.tile([128, 128], F32, tag="C")
                nc.tensor.transpose(pst[:], hraT[:, t * 128:(t + 1) * 128], ident[:])
                hb = smallp.tile([128, 128], BF16, tag="hrmbf")
                topk_mask_mul(pst[:], hb[:])
                hrm_bf.append(hb)

            # =========== BN0 (local, full B) ===========
            def bn_finalize(stats_ap, g_tile, b_tile):
                mv = smallp.tile([128, 2], F32, tag="mv")
                nc.vector.bn_aggr(mv[:], stats_ap)
                scale = smallp.tile([128, 1], F32, tag="sc")
                shift = smallp.tile([128, 1], F32, tag="sh")
                tmp = smallp.tile([128, 1], F32, tag="tm")
                nc.vector.tensor_scalar_add(tmp[:], mv[:, 1:2], 1e-5)
                nc.scalar.activation(scale[:], tmp[:], AF.Sqrt)
                nc.vector.reciprocal(scale[:], scale[:])
                nc.vector.tensor_mul(scale[:], scale[:], g_tile[:, 0:1])
                nc.vector.tensor_mul(tmp[:], mv[:, 0:1], scale[:])
                nc.vector.tensor_sub(shift[:], b_tile[:, 0:1], tmp[:])
                return scale, shift

            st0 = smallp.tile([128, 4, 6], F32, tag="st0")
            for i in range(4):
                nc.vector.bn_stats(st0[:, i, :], hsT_full[:, i * 512:(i + 1) * 512])
            bn0_scale, bn0_shift = bn_finalize(st0[:], bn0g_s, bn0b_s)

            # ha (bn0-applied hs shard) in [b, c] bf16 tiles
            haT_bf = smallp.tile([128, BSH], BF16, tag="haTbf")
            nc.vector.tensor_scalar(haT_bf[:], hsT_full[:, 0:BSH], bn0_scale[:, 0:1],
                                    bn0_shift[:, 0:1], op0=ALU.mult, op1=ALU.add)
            ha_bf = []
            for t in range(2):
                pst = psC.tile([128, 128], BF16, tag="Cb")
                nc.tensor.transpose(pst[:], haT_bf[:, t * 128:(t + 1) * 128],
                                    ident_bf[:])
                hb = smallp.tile([128, 128], BF16, tag="habf")
                evac(hb[:], pst[:])
                ha_bf.append(hb)

            # Wm / inter shard accumulators in [b, d] layout
            Wm_sh = [smallp.tile([128, 128], BF16, tag=f"wmsh{t}",
                                 name=f"Wm_sh{t}") for t in range(2)]
            inter_sh = [smallp.tile([128, 128], BF16, tag=f"ish{t}",
                                    name=f"inter_sh{t}") for t in range(2)]
            idx_t = []
            for t in range(2):
                idx = smallp.tile([128, 2], I32, tag=f"idx{t}")
                nc.sync.dma_start(out=idx[:], in_=relidx[t * 128:(t + 1) * 128, :])
                idx_t.append(idx)

            # =========== tail weights ===========
            tsw1_t = load_w1(tsw1, 4, "w1ts")
            taw1_t = load_w1(taw1, 4, "w1ta")

            # ---------------- W 4-chunk group (DVE multiply + reduce) ----------------
            def w_group(gi):
                t, q = divmod(gi, 8)          # q = which 4-chunk quarter (d0 = 16q)
                wsb = wsbp.tile([128, 4, 4, 128], BF16, tag="wsb")
                for jj in range(4):
                    j = q * 4 + jj            # chunk index 0..31 (d = 4j..4j+3)
                    hsel, off = divmod(j * 512, HALF)
                    pw = psB.tile([128, 512], F32, tag="B")
                    mm(pw[:], rsT_bf[:, t * 128:(t + 1) * 128],
                       core_h[hsel][:, off:off + 512])
                    # DVE multiply straight from PSUM, write bf16 SBUF
                    nc.vector.tensor_tensor(
                        out=wsb[:, jj, :, :],
                        in0=pw[:].rearrange("p (d c) -> p d c", c=128),
                        in1=ha_bf[t][:, None, :].to_broadcast([128, 4, 128]),
                        op=ALU.mult)
                w4 = wsb[:].rearrange("p j d c -> p (j d) c")
                with nc.allow_low_precision("bf16 Wm accumulate"):
                    nc.vector.tensor_reduce(Wm_sh[t][:, 16 * q:16 * (q + 1)],
                                            w4, axis=AX.X, op=ALU.add)

            # ---------------- inter tile (t, h) ----------------
            def inter_tile(t, h):
                pc = pcp.tile([128, HALF], BF16, tag="pc")
                nc.gpsimd.indirect_dma_start(
                    out=pc[:], out_offset=None, in_=cbT[:],
                    in_offset=IndirectOffsetOnAxis(ap=idx_t[t][:, h:h + 1], axis=0))
                nc.scalar.activation(pc[:], pc[:], AF.Tanh)
                pc3 = pc[:].rearrange("p (d c) -> p d c", c=128)
                nc.gpsimd.tensor_tensor(
                    out=pc3, in0=pc3,
                    in1=hrm_bf[t][:, None, :].to_broadcast([128, 64, 128]),
                    op=ALU.mult)
                with nc.allow_low_precision("bf16 inter accumulate"):
                    nc.vector.tensor_reduce(inter_sh[t][:, h * 64:(h + 1) * 64],
                                            pc3, axis=AX.X, op=ALU.add)

            # ---------------- tail MLP group ----------------
            def tail_group(g):
                xt_g = load_xt(tailT, g * 512, 512, 4)
                h1t = mlp_l1(tsw1_t, bias["tsb1"], xt_g, 512)
                mlp_l2_T(tsw2_s, bias["tsb2"], h1t, 512,
                         tsT_s[:, g * 512:(g + 1) * 512])
                h1a = mlp_l1(taw1_t, bias["tab1"], xt_g, 512)
                for nt in range(4):
                    pt = psC.tile([128, 128], F32, tag="C")
                    for m in range(4):
                        mm(pt[:], h1a[:, m, nt * 128:(nt + 1) * 128],
                           taw2_s[:, m, :], start=(m == 0), stop=False)
                    mm(pt[:], ones_row[0:1, 0:128], bias["tab2"][0:1, :],
                       start=False, stop=True)
                    tam_nc = smallp.tile([128, 128], BF16, tag="tamnc")
                    topk_mask_mul(pt[:], tam_nc[:])
                    ptT = psC.tile([128, 128], BF16, tag="Cb")
                    nc.tensor.transpose(ptT[:], tam_nc[:], ident_bf[:])
                    evac(tamT_s[:, g * 512 + nt * 128:g * 512 + (nt + 1) * 128],
                         ptT[:], which="a")

            # =========== Phase 2: Wm/inter chains front-loaded, tails fill ======
            tail_group(0)
            for t in range(2):
                for h in range(2):
                    inter_tile(t, h)
            for gi in range(8):
                w_group(gi)
            tail_group(1)
            for gi in range(8, 16):
                w_group(gi)
            tail_group(2)

            # =========== Phase 3: pack shard, AllGather (overlaps tails 3-4) ====
            for t in range(2):
                pst = psC.tile([128, 128], BF16, tag="Cb")
                nc.tensor.transpose(pst[:], Wm_sh[t][:], ident_bf[:])
                evac(WmT_sh[:, t * 128:(t + 1) * 128], pst[:])
                pst2 = psC.tile([128, 128], BF16, tag="Cb")
                nc.tensor.transpose(pst2[:], inter_sh[t][:], ident_bf[:])
                evac(intT_sh[:, t * 128:(t + 1) * 128], pst2[:])

            ag2_in = dramp.tile([2, 128, BSH], BF16)
            ag2_out = dramp.tile([NCORES, 2, 128, BSH], BF16, addr_space="Shared")
            nc.sync.dma_start(out=ag2_in[0], in_=WmT_sh[:])
            nc.sync.dma_start(out=ag2_in[1], in_=intT_sh[:])
            nc.gpsimd.collective_compute(
                "AllGather", ALU.bypass,
                replica_groups=[list(range(NCORES))],
                ins=[ag2_in.opt()], outs=[ag2_out.opt()])

            tail_group(3)
            tail_group(4)
            psC.release()

            nc.sync.dma_start(
                out=WmT_all[:], in_=ag2_out[:, 0].rearrange("r d b -> d r b"))
            nc.sync.dma_start(
                out=intT_all[:], in_=ag2_out[:, 1].rearrange("r d b -> d r b"))

            # BN1 on gathered WmT (full B)
            st1 = smallp.tile([128, 4, 6], F32, tag="st1")
            for i in range(4):
                nc.vector.bn_stats(st1[:, i, :], WmT_all[:, i * 512:(i + 1) * 512])
            bn1_scale, bn1_shift = bn_finalize(st1[:], bn1g_s, bn1b_s)
            nc.vector.tensor_scalar(WmT_nb[:], WmT_all[:], bn1_scale[:, 0:1],
                                    bn1_shift[:, 0:1], op0=ALU.mult, op1=ALU.add)

            # =========== Phase 4: scores (quad ring psA/psA/psD = depth 3) ======
            psD = tc.alloc_tile_pool(name="psD", bufs=1, space="PSUM")
            NLAST = NSH - 4 * 512               # 452 valid cols in group 4
            quad_i = [0]

            def quad_tile():
                quad_i[0] += 1
                pool = psD if quad_i[0] % 3 == 0 else psA
                return pool.tile([128, 2, 512], F32, tag="D" if pool is psD else "A",
                                 name=f"psq{quad_i[0]}")

            for bt in range(NB_FULL):
                r0 = bt * 128
                for lhsT, outd, rhs in ((WmT_nb, tucker, tsT_s),
                                        (intT_all, poss, tamT_s)):
                    for half in range(2):
                        psq = quad_tile()
                        for gg in range(2):
                            g = half * 2 + gg
                            mm(psq[:, gg, :], lhsT[:, r0:r0 + 128],
                               rhs[:, g * 512:(g + 1) * 512])
                        stq = stagep.tile([128, 1024], BF16, tag="stq")
                        evac(stq[:], psq[:].rearrange("p g n -> p (g n)"))
                        nc.sync.dma_start(
                            out=outd[r0:r0 + 128, half * 1024:(half + 1) * 1024],
                            in_=stq[:])
                    ps4 = psB.tile([128, 512], F32, tag="B")
                    mm(ps4[:], lhsT[:, r0:r0 + 128], rhs[:, 4 * 512:5 * 512])
                    st4 = stagep.tile([128, 512], BF16, tag="st4")
                    evac(st4[:], ps4[:])
                    nc.sync.dma_start(out=outd[r0:r0 + 128, 2048:NSH],
                                      in_=st4[:, 0:NLAST])
            psD.release()

    nc.finalize()
    return nc


# ---------------------------------------------------------------------------
# host side
# ---------------------------------------------------------------------------

BF = np.dtype(ml_dtypes.bfloat16)


def _to_np(x, dt=np.float32):
    return np.ascontiguousarray(np.asarray(x), dtype=dt)


def prepare_in_maps(inputs):
    head = _to_np(inputs["head_vector"])        # [B, E]
    rel = _to_np(inputs["relation_vector"])     # [B, E]
    ridx = np.ascontiguousarray(np.asarray(inputs["relation_index"]).astype(np.int32))
    tailv = _to_np(inputs["tail_vector"])       # [N, E]
    codebook = _to_np(inputs["codebook"])       # [R2, C, C]
    core = _to_np(inputs["core"])               # [C, C, C]

    cbT_host = np.ascontiguousarray(
        codebook.transpose(0, 2, 1).reshape(2 * R2, C * C // 2)).astype(BF)
    core_dc_host = np.ascontiguousarray(
        core.transpose(0, 2, 1).reshape(C, C * C)).astype(BF)

    headT_full = np.ascontiguousarray(head.T).astype(BF)   # [E, B]
    relT_full = np.ascontiguousarray(rel.T).astype(BF)     # [E, B]
    tailT_full = np.ascontiguousarray(tailv.T).astype(BF)  # [E, N]

    def chunked_bias(k, nk):
        return np.ascontiguousarray(_to_np(inputs[k]).reshape(nk, 128).T)

    col = lambda k: _to_np(inputs[k]).reshape(128, 1)
    wcast = lambda k: _to_np(inputs[k]).astype(BF)
    weights_common = {
        "hsw1": wcast("hsw1"), "hsb1": chunked_bias("hsb1", 4),
        "hsw2": wcast("hsw2"), "hsb2": col("hsb2"),
        "rsw1": wcast("rsw1"), "rsb1": chunked_bias("rsb1", 4),
        "rsw2": wcast("rsw2"), "rsb2": col("rsb2"),
        "tsw1": wcast("tsw1"), "tsb1": chunked_bias("tsb1", 4),
        "tsw2": wcast("tsw2"), "tsb2": col("tsb2"),
        "taw1": wcast("taw1"), "tab1": chunked_bias("tab1", 4),
        "taw2": wcast("taw2"),
        "tab2": _to_np(inputs["tab2"]).reshape(1, C).astype(BF),
        "hrw1": wcast("hrw1"), "hrb1": chunked_bias("hrb1", 2),
        "hrw2": wcast("hrw2"), "hrb2": chunked_bias("hrb2", 2),
        "hrw3": wcast("hrw3"), "hrb3": col("hrb3"),
        "bn0g": col("bn0_g"), "bn0b": col("bn0_b"),
        "bn1g": col("bn1_g"), "bn1b": col("bn1_b"),
        "cbT": cbT_host, "core_dc": core_dc_host,
    }

    in_maps = []
    for k in range(NCORES):
        b0 = k * BSH
        n0 = k * NSH
        # rotate headT so THIS core's 256 b-columns come first; BN0 stats are
        # order-invariant and slices [0:256] are "my" shard on every core.
        headT_k = np.ascontiguousarray(np.roll(headT_full, -b0, axis=1))
        tailT_k = np.zeros((E, NPAD), BF)
        tailT_k[:, :NSH] = tailT_full[:, n0:n0 + NSH]
        m = dict(weights_common)
        m["headT"] = headT_k
        m["relT"] = np.ascontiguousarray(relT_full[:, b0:b0 + BSH])
        m["tailT"] = tailT_k
        ri = ridx[b0:b0 + BSH]
        m["relidx"] = np.ascontiguousarray(
            np.stack([2 * ri, 2 * ri + 1], axis=1))
        in_maps.append(m)
    return in_maps


def assemble_outputs(results):
    tuckers, posses = [], []
    for k in range(NCORES):
        r = results[k]
        tuckers.append(np.asarray(r["tucker"]).astype(np.float32))
        posses.append(np.asarray(r["poss"]).astype(np.float32))
    tucker_full = np.concatenate(tuckers, axis=1)
    poss_full = np.concatenate(posses, axis=1)
    return tucker_full, poss_full


def kernel(**inputs):
    if "prog" not in _PROG_CACHE:
        _PROG_CACHE["prog"] = build_program()
    nc = _PROG_CACHE["prog"]
    in_maps = prepare_in_maps(inputs)
    res = run_bass_kernel_spmd(nc, in_maps, list(range(NCORES)))
    return assemble_outputs(res.results)


# revision 25
# speedup vs baseline: 1.3910x; 1.0430x over previous
"""Trainium2 Bass kernel for nn_BaseModel_74302934220896 (TuckER + possibility-codebook).

Contract: kernel(**inputs) takes FULL unsharded inputs (as in reference.setup_inputs())
and returns the full output tuple (tucker_logits [B,N] f32, possibility_score [B,N] f32).

Sharding (8 cores):
  - B (2048) -> 8 x 256 for relation/hr/codebook-gather paths
  - N (20000) -> 8 x 2500 (padded to 2560) for tail features and the [B,N] score matmuls
  - head MLP replicated over full B on every core so BN0 needs no collective
  - ONE AllGather carries the per-core [WmT(bf16); interT(bf16)] shards; BN1 stats
    computed locally from the gathered full-B WmT.

v3 design notes:
  - all matmul inputs bf16; outputs bf16 (host upcasts)
  - MLP biases folded into the PSUM->SBUF evacuation (ACT Relu/Identity with
    per-partition bias AP, or DVE scalar_tensor_tensor) -- no separate bias ops
  - Wm / inter einsums ('bc,bcd->bd'): broadcast tensor_tensor multiply
    (alternating GPSIMD-from-SBUF and DVE-from-PSUM paths) + wide last-axis
    tensor_reduce on DVE with bf16 accumulation
  - ta computed directly in [n, c] layout so topk masking needs no pre-transpose
  - score PSUM evacuated in [128,1024] pair-tiles, round-robin DVE/ACT
"""

import sys

sys.path.insert(0, "/opt/trn_rl_repo")

import numpy as np
import ml_dtypes

import concourse.bass as bass
import concourse.bacc as bacc
import concourse.mybir as mybir
import concourse.tile as tile
from concourse.bass import IndirectOffsetOnAxis
from concourse.bass_utils import run_bass_kernel_spmd
from concourse.masks import make_identity

F32 = mybir.dt.float32
BF16 = mybir.dt.bfloat16
I32 = mybir.dt.int32
AF = mybir.ActivationFunctionType
ALU = mybir.AluOpType
AX = mybir.AxisListType

B, N, E, C, R2 = 2048, 20000, 512, 128, 474
NCORES = 8
BSH = B // NCORES            # 256 b rows per core
NSH = N // NCORES            # 2500 tail rows per core
NPAD = 2560                  # padded to 5 groups of 512
NG = NPAD // 512             # 5 n-groups
NB_FULL = B // 128           # 16 b-tiles over full B
TEMP = 0.5
NEG = -1.0e30

_PROG_CACHE = {}


def build_program():
    nc = bacc.Bacc("TRN2", target_bir_lowering=False, debug=False,
                   num_devices=NCORES)

    # ---------------- DRAM I/O ----------------
    dI = lambda name, shape, dt=BF16: nc.dram_tensor(name, shape, dt, kind="ExternalInput")
    headT = dI("headT", [E, B])                    # full-B head_vector^T (rolled)
    relT = dI("relT", [E, BSH])                    # sharded relation_vector^T
    tailT = dI("tailT", [E, NPAD])                 # sharded+padded tail_vector^T
    relidx = dI("relidx", [BSH, 2], I32)
    cbT = dI("cbT", [2 * R2, C * C // 2])          # tanh-input codebook, rows (d-half, c)
    core_dc = dI("core_dc", [C, C * C])            # core as [e, (d, c)] (c fastest)

    hsw1 = dI("hsw1", [E, E]); rsw1 = dI("rsw1", [E, E])
    tsw1 = dI("tsw1", [E, E]); taw1 = dI("taw1", [E, E])
    hsw2 = dI("hsw2", [E, C]); rsw2 = dI("rsw2", [E, C])
    tsw2 = dI("tsw2", [E, C]); taw2 = dI("taw2", [E, C])
    hrw1 = dI("hrw1", [2 * E, 2 * C])
    hrw2 = dI("hrw2", [2 * C, 2 * C])
    hrw3 = dI("hrw3", [2 * C, C])
    # l1 biases chunked [128, nk] f32; l2 biases [128, 1] f32; tab2 row [1, C]
    hsb1 = dI("hsb1", [128, 4], F32); hsb2 = dI("hsb2", [128, 1], F32)
    rsb1 = dI("rsb1", [128, 4], F32); rsb2 = dI("rsb2", [128, 1], F32)
    tsb1 = dI("tsb1", [128, 4], F32); tsb2 = dI("tsb2", [128, 1], F32)
    tab1 = dI("tab1", [128, 4], F32); tab2 = dI("tab2", [1, C])
    hrb1 = dI("hrb1", [128, 2], F32); hrb2 = dI("hrb2", [128, 2], F32)
    hrb3 = dI("hrb3", [128, 1], F32)
    bn0g = dI("bn0g", [128, 1], F32); bn0b = dI("bn0b", [128, 1], F32)
    bn1g = dI("bn1g", [128, 1], F32); bn1b = dI("bn1b", [128, 1], F32)

    tucker = nc.dram_tensor("tucker", [B, NSH], BF16, kind="ExternalOutput")
    poss = nc.dram_tensor("poss", [B, NSH], BF16, kind="ExternalOutput")

    with tile.TileContext(nc) as tc:
        with (
            tc.tile_pool(name="const", bufs=1) as constp,
            tc.tile_pool(name="w1p", bufs=4) as w1p,         # 4 bufs per family tag
            tc.tile_pool(name="hrw1p", bufs=8) as hrw1p,
            tc.tile_pool(name="w2p", bufs=1) as w2p,
            tc.tile_pool(name="big", bufs=1) as bigp,        # core_dc halves (2 tags)
            tc.tile_pool(name="pcp", bufs=2) as pcp,         # gathered codebook tiles
            tc.tile_pool(name="xt", bufs=8) as xtp,
            tc.tile_pool(name="h1", bufs=2) as h1p,
            tc.tile_pool(name="wsb", bufs=2) as wsbp,        # W 4-chunk sbuf tiles
            tc.tile_pool(name="pers", bufs=1) as pers,
            tc.tile_pool(name="small", bufs=2) as smallp,
            tc.tile_pool(name="stage", bufs=3) as stagep,
            tc.tile_pool(name="psA", bufs=2, space="PSUM") as psA,   # [128,2,512] = 2 banks
            tc.tile_pool(name="psB", bufs=2, space="PSUM") as psB,   # [128,512]   = 1 bank
            tc.tile_pool(name="dram", bufs=1, space="DRAM") as dramp,
        ):
            # psC lives through phases 1-3, then is released so psD (scores
            # pipeline depth-3) can use its banks in phase 4.
            psC = tc.alloc_tile_pool(name="psC", bufs=1, space="PSUM")
            # ---------------- constants ----------------
            ident = constp.tile([128, 128], F32)
            make_identity(nc, ident[:])
            ident_bf = constp.tile([128, 128], BF16, tag="idbf")
            nc.gpsimd.tensor_copy(ident_bf[:], ident[:])
            ones_row = constp.tile([1, 512], BF16, tag="ones")
            nc.gpsimd.memset(ones_row[:], 1.0)
            zeros_t = constp.tile([128, 512], BF16, tag="zeros")
            nc.gpsimd.memset(zeros_t[:], 0.0)

            def load_const(dram_t, shape, tag, dt=F32):
                t = constp.tile(shape, dt, tag=tag)
                nc.sync.dma_start(out=t[:], in_=dram_t[:])
                return t

            # biases/weights are loaded lazily, right before their first user,
            # so the head MLP's inputs lead the DMA queue.
            bias = {}
            _bias_spec = {
                "hsb1": (hsb1, [128, 4], F32), "hsb2": (hsb2, [128, 1], F32),
                "rsb1": (rsb1, [128, 4], F32), "rsb2": (rsb2, [128, 1], F32),
                "tsb1": (tsb1, [128, 4], F32), "tsb2": (tsb2, [128, 1], F32),
                "tab1": (tab1, [128, 4], F32), "tab2": (tab2, [1, C], BF16),
                "hrb1": (hrb1, [128, 2], F32), "hrb2": (hrb2, [128, 2], F32),
                "hrb3": (hrb3, [128, 1], F32)}

            def load_bias(*names):
                for nm in names:
                    dr, shp, dt = _bias_spec[nm]
                    bias[nm] = load_const(dr, shp, "b_" + nm, dt)

            def load_w1(w1_dram, nk, tag, pool=None):
                pool = pool or w1p
                w1_t = []
                for k in range(nk):
                    wt = pool.tile([128, w1_dram.shape[1]], BF16, tag=tag)
                    nc.sync.dma_start(out=wt[:], in_=w1_dram[k * 128:(k + 1) * 128, :])
                    w1_t.append(wt)
                return w1_t

            def load_w2(w, nk, tag):
                t = w2p.tile([128, nk, w.shape[1]], BF16, tag=tag)
                nc.sync.dma_start(
                    out=t[:], in_=w[:].rearrange("(k p) c -> p k c", p=128))
                return t

            HALF = C * C // 2

            # persistent tiles
            tsT_s = pers.tile([128, NPAD], BF16)      # ts^T (+biases)
            tamT_s = pers.tile([128, NPAD], BF16)     # tam^T
            WmT_all = pers.tile([128, B], BF16)       # gathered Wm^T raw
            intT_all = pers.tile([128, B], BF16)      # gathered inter^T
            WmT_nb = pers.tile([128, B], BF16)        # BN1-applied, score lhsT
            hsT_full = pers.tile([128, B], F32)       # hs^T full B (pre-BN)
            rsT_bf = pers.tile([128, BSH], BF16)
            WmT_sh = pers.tile([128, BSH], BF16)
            intT_sh = pers.tile([128, BSH], BF16)

            # ---------------- engine routing ----------------
            evac_i = [0]

            def evac(out_ap, ps_ap, which=None):
                """PSUM->SBUF copy routed round-robin DVE/ACT."""
                if which is None:
                    evac_i[0] += 1
                    which = "v" if evac_i[0] % 2 else "a"
                if which == "v":
                    nc.vector.tensor_copy(out_ap, ps_ap)
                else:
                    nc.scalar.activation(out_ap, ps_ap, AF.Copy)

            relu_i = [0]

            def relu_bias_evac(out_ap, ps_ap, bias_col):
                """out = relu(ps + bias), [128, nb]; ACT-biased routing (2a:1v)."""
                relu_i[0] += 1
                if relu_i[0] % 3 == 0:
                    nb = ps_ap.shape[-1]
                    nc.vector.scalar_tensor_tensor(
                        out_ap, ps_ap, bias_col, zeros_t[:, 0:nb],
                        op0=ALU.add, op1=ALU.max)
                else:
                    nc.scalar.activation(out_ap, ps_ap, AF.Relu, bias=bias_col)

            def bias_evac(out_ap, ps_ap, bias_col):
                """out = ps + bias (per-partition); ACT-biased routing."""
                evac_i[0] += 1
                if evac_i[0] % 3 == 0:
                    nc.vector.tensor_scalar_add(out_ap, ps_ap, bias_col)
                else:
                    nc.scalar.activation(out_ap, ps_ap, AF.Identity, bias=bias_col)

            def mm(out, lhsT, rhs, start=True, stop=True):
                nc.tensor.matmul(out, lhsT, rhs, start=start, stop=stop)

            # ---------------- MLP helpers ----------------
            def load_xt(xT_dram, col0, nb, nk):
                xt_t = []
                for k in range(nk):
                    xt = xtp.tile([128, nb], BF16, tag=f"xt{nb}")
                    nc.sync.dma_start(
                        out=xt[:], in_=xT_dram[k * 128:(k + 1) * 128, col0:col0 + nb])
                    xt_t.append(xt)
                return xt_t

            def mlp_l1(w1_t, b1c, xt_t, nb, nm=4):
                """h1 [128, nm, nb] bf16 = relu(x @ w1 + b1)^T in m-chunks."""
                h1 = h1p.tile([128, nm, nb], BF16, tag=f"h1_{nm}_{nb}")
                for half in range(nm // 2):
                    ps = psA.tile([128, 2, nb], F32, tag="A")
                    for mi in range(2):
                        m = half * 2 + mi
                        for k in range(len(w1_t)):
                            mm(ps[:, mi, :], w1_t[k][:, m * 128:(m + 1) * 128],
                               xt_t[k][:], start=(k == 0), stop=(k == len(w1_t) - 1))
                        relu_bias_evac(h1[:, half * 2 + mi, :], ps[:, mi, :],
                                       b1c[:, m:m + 1])
                return h1

            def mlp_l2_T(w2_s, b2c, h1, nb, out_ap, nm=4):
                """out [128 c, nb] = (h1 @ w2 + b2)^T."""
                ps2 = psB.tile([128, nb], F32, tag="B")
                for m in range(nm):
                    mm(ps2[:], w2_s[:, m, :], h1[:, m, :], start=(m == 0),
                       stop=(m == nm - 1))
                bias_evac(out_ap, ps2[:], b2c[:, 0:1])

            # ---------- soft top-10 mask ([128,128] f32 AP, may be PSUM) ----------
            def topk_mask_mul(x_ap, out_ap):
                """out = sigmoid((x - thr10)/TEMP) * x   (out bf16)"""
                m8 = smallp.tile([128, 8], F32, tag="m8")
                zap = smallp.tile([128, 128], F32, tag="zap")
                nc.vector.max(out=m8[:], in_=x_ap)
                nc.vector.match_replace(out=zap[:], in_to_replace=m8[:],
                                        in_values=x_ap, imm_value=NEG)
                nc.vector.max(out=m8[:], in_=zap[:])
                thr = smallp.tile([128, 1], F32, tag="thr")
                nc.vector.tensor_scalar_mul(thr[:], m8[:, 1:2], -1.0 / TEMP)
                mask = smallp.tile([128, 128], F32, tag="mask")
                nc.scalar.activation(mask[:], x_ap, AF.Sigmoid,
                                     bias=thr[:, 0:1], scale=1.0 / TEMP)
                nc.vector.tensor_mul(out_ap, mask[:], x_ap)

            # =========== Phase 1: head MLP (full B) ===========
            load_bias("hsb1", "hsb2")
            hsw1_t = load_w1(hsw1, 4, "w1hs")
            hsw2_s = load_w2(hsw2, 4, "w2a")
            for bg in range(B // 512):
                xt_bg = load_xt(headT, bg * 512, 512, 4)
                h1h = mlp_l1(hsw1_t, bias["hsb1"], xt_bg, 512)
                mlp_l2_T(hsw2_s, bias["hsb2"], h1h, 512,
                         hsT_full[:, bg * 512:(bg + 1) * 512])

            # =========== rel MLP ===========
            load_bias("rsb1", "rsb2")
            xt_r = load_xt(relT, 0, BSH, 4)
            rsw1_t = load_w1(rsw1, 4, "w1rs")
            rsw2_s = load_w2(rsw2, 4, "w2b")
            h1r = mlp_l1(rsw1_t, bias["rsb1"], xt_r, BSH)
            mlp_l2_T(rsw2_s, bias["rsb2"], h1r, BSH, rsT_bf[:])

            # =========== hr MLP (3 layers) ===========
            load_bias("hrb1", "hrb2", "hrb3")
            hrw1_t = load_w1(hrw1, 8, "w1hr", hrw1p)
            hrw2_s = load_w2(hrw2, 2, "w2f")
            hrw3_s = load_w2(hrw3, 2, "w2e")
            hr_x = load_xt(headT, 0, BSH, 4) + xt_r   # concat(head_my, rel)
            hrh1 = h1p.tile([128, 2, BSH], BF16, tag="hrh1")
            ps = psA.tile([128, 2, BSH], F32, tag="A")
            for mi in range(2):
                for k in range(8):
                    mm(ps[:, mi, :], hrw1_t[k][:, mi * 128:(mi + 1) * 128],
                       hr_x[k][:], start=(k == 0), stop=(k == 7))
                relu_bias_evac(hrh1[:, mi, :], ps[:, mi, :], bias["hrb1"][:, mi:mi + 1])
            hrh2 = h1p.tile([128, 2, BSH], BF16, tag="hrh2")
            ps = psA.tile([128, 2, BSH], F32, tag="A")
            for mi in range(2):
                for k in range(2):
                    mm(ps[:, mi, :], hrw2_s[:, k, mi * 128:(mi + 1) * 128],
                       hrh1[:, k, :], start=(k == 0), stop=(k == 1))
                relu_bias_evac(hrh2[:, mi, :], ps[:, mi, :], bias["hrb2"][:, mi:mi + 1])
            hraT = smallp.tile([128, BSH], F32, tag="hraT")
            ps2 = psB.tile([128, BSH], F32, tag="B")
            for k in range(2):
                mm(ps2[:], hrw3_s[:, k, :], hrh2[:, k, :], start=(k == 0),
                   stop=(k == 1))
            bias_evac(hraT[:], ps2[:], bias["hrb3"][:, 0:1])

            # hrm (masked hra) in [b, c] layout, bf16, per t-tile
            hrm_bf = []
            for t in range(2):
                pst = psC.tile([128, 128], F32, tag="C")
                nc.tensor.transpose(pst[:], hraT[:, t * 128:(t + 1) * 128], ident[:])
                hb = smallp.tile([128, 128], BF16, tag="hrmbf")
                topk_mask_mul(pst[:], hb[:])
                hrm_bf.append(hb)

            # =========== BN0 (local, full B) ===========
            def bn_finalize(stats_ap, g_tile, b_tile):
                mv = smallp.tile([128, 2], F32, tag="mv")
                nc.vector.bn_aggr(mv[:], stats_ap)
                scale = smallp.tile([128, 1], F32, tag="sc")
                shift = smallp.tile([128, 1], F32, tag="sh")
                tmp = smallp.tile([128, 1], F32, tag="tm")
                nc.vector.tensor_scalar_add(tmp[:], mv[:, 1:2], 1e-5)
                nc.scalar.activation(scale[:], tmp[:], AF.Sqrt)
                nc.vector.reciprocal(scale[:], scale[:])
                nc.vector.tensor_mul(scale[:], scale[:], g_tile[:, 0:1])
                nc.vector.tensor_mul(tmp[:], mv[:, 0:1], scale[:])
                nc.vector.tensor_sub(shift[:], b_tile[:, 0:1], tmp[:])
                return scale, shift

            bn0g_s = load_const(bn0g, [128, 1], "c11")
            bn0b_s = load_const(bn0b, [128, 1], "c12")
            st0 = smallp.tile([128, 4, 6], F32, tag="st0")
            for i in range(4):
                nc.vector.bn_stats(st0[:, i, :], hsT_full[:, i * 512:(i + 1) * 512])
            bn0_scale, bn0_shift = bn_finalize(st0[:], bn0g_s, bn0b_s)

            # ha (bn0-applied hs shard) in [b, c] bf16 tiles
            haT_bf = smallp.tile([128, BSH], BF16, tag="haTbf")
            nc.vector.tensor_scalar(haT_bf[:], hsT_full[:, 0:BSH], bn0_scale[:, 0:1],
                                    bn0_shift[:, 0:1], op0=ALU.mult, op1=ALU.add)
            ha_bf = []
            for t in range(2):
                pst = psC.tile([128, 128], BF16, tag="Cb")
                nc.tensor.transpose(pst[:], haT_bf[:, t * 128:(t + 1) * 128],
                                    ident_bf[:])
                hb = smallp.tile([128, 128], BF16, tag="habf")
                evac(hb[:], pst[:])
                ha_bf.append(hb)

            # Wm / inter shard accumulators in [b, d] layout
            Wm_sh = [smallp.tile([128, 128], BF16, tag=f"wmsh{t}",
                                 name=f"Wm_sh{t}") for t in range(2)]
            inter_sh = [smallp.tile([128, 128], BF16, tag=f"ish{t}",
                                    name=f"inter_sh{t}") for t in range(2)]
            idx_t = []
            for t in range(2):
                idx = smallp.tile([128, 2], I32, tag=f"idx{t}")
                nc.sync.dma_start(out=idx[:], in_=relidx[t * 128:(t + 1) * 128, :])
                idx_t.append(idx)

            # =========== tail weights + core (for w_groups) ===========
            load_bias("tsb1", "tsb2", "tab1", "tab2")
            tsw1_t = load_w1(tsw1, 4, "w1ts")
            taw1_t = load_w1(taw1, 4, "w1ta")
            tsw2_s = load_w2(tsw2, 4, "w2c")
            taw2_s = load_w2(taw2, 4, "w2d")
            core_h = []
            for h in range(2):
                ct = bigp.tile([128, HALF], BF16, tag=f"core{h}", name=f"core{h}")
                nc.sync.dma_start(out=ct[:], in_=core_dc[:, h * HALF:(h + 1) * HALF])
                core_h.append(ct)

            # ---------------- W 4-chunk group (DVE multiply + reduce) ----------------
            def w_group(gi):
                t, q = divmod(gi, 8)          # q = which 4-chunk quarter (d0 = 16q)
                wsb = wsbp.tile([128, 4, 4, 128], BF16, tag="wsb")
                for jj in range(4):
                    j = q * 4 + jj            # chunk index 0..31 (d = 4j..4j+3)
                    hsel, off = divmod(j * 512, HALF)
                    pw = psB.tile([128, 512], F32, tag="B")
                    mm(pw[:], rsT_bf[:, t * 128:(t + 1) * 128],
                       core_h[hsel][:, off:off + 512])
                    # DVE multiply straight from PSUM, write bf16 SBUF
                    nc.vector.tensor_tensor(
                        out=wsb[:, jj, :, :],
                        in0=pw[:].rearrange("p (d c) -> p d c", c=128),
                        in1=ha_bf[t][:, None, :].to_broadcast([128, 4, 128]),
                        op=ALU.mult)
                w4 = wsb[:].rearrange("p j d c -> p (j d) c")
                with nc.allow_low_precision("bf16 Wm accumulate"):
                    nc.vector.tensor_reduce(Wm_sh[t][:, 16 * q:16 * (q + 1)],
                                            w4, axis=AX.X, op=ALU.add)

            # ---------------- inter tile (t, h) ----------------
            def inter_tile(t, h):
                pc = pcp.tile([128, HALF], BF16, tag="pc")
                nc.gpsimd.indirect_dma_start(
                    out=pc[:], out_offset=None, in_=cbT[:],
                    in_offset=IndirectOffsetOnAxis(ap=idx_t[t][:, h:h + 1], axis=0))
                nc.scalar.activation(pc[:], pc[:], AF.Tanh)
                pc3 = pc[:].rearrange("p (d c) -> p d c", c=128)
                nc.gpsimd.tensor_tensor(
                    out=pc3, in0=pc3,
                    in1=hrm_bf[t][:, None, :].to_broadcast([128, 64, 128]),
                    op=ALU.mult)
                # tree-reduce over c (bf16 TT adds run 2x; tensor_reduce is 1x)
                w = 64
                while w >= 1:
                    nc.vector.tensor_tensor(
                        out=pc3[:, :, 0:w], in0=pc3[:, :, 0:w],
                        in1=pc3[:, :, w:2 * w], op=ALU.add)
                    w //= 2
                nc.vector.tensor_copy(inter_sh[t][:, h * 64:(h + 1) * 64],
                                      pc3[:, :, 0])

            # ---------------- tail MLP group ----------------
            def tail_group(g):
                xt_g = load_xt(tailT, g * 512, 512, 4)
                h1t = mlp_l1(tsw1_t, bias["tsb1"], xt_g, 512)
                mlp_l2_T(tsw2_s, bias["tsb2"], h1t, 512,
                         tsT_s[:, g * 512:(g + 1) * 512])
                h1a = mlp_l1(taw1_t, bias["tab1"], xt_g, 512)
                for nt in range(4):
                    pt = psC.tile([128, 128], F32, tag="C")
                    for m in range(4):
                        mm(pt[:], h1a[:, m, nt * 128:(nt + 1) * 128],
                           taw2_s[:, m, :], start=(m == 0), stop=False)
                    mm(pt[:], ones_row[0:1, 0:128], bias["tab2"][0:1, :],
                       start=False, stop=True)
                    tam_nc = smallp.tile([128, 128], BF16, tag="tamnc")
                    topk_mask_mul(pt[:], tam_nc[:])
                    ptT = psC.tile([128, 128], BF16, tag="Cb")
                    nc.tensor.transpose(ptT[:], tam_nc[:], ident_bf[:])
                    evac(tamT_s[:, g * 512 + nt * 128:g * 512 + (nt + 1) * 128],
                         ptT[:], which="a")

            # =========== Phase 2: Wm/inter chains front-loaded, tails fill ======
            tail_group(0)
            for t in range(2):
                for h in range(2):
                    inter_tile(t, h)
            for gi in range(8):
                w_group(gi)
            tail_group(1)
            for gi in range(8, 16):
                w_group(gi)

            # =========== Phase 3: pack shard, AllGather (overlaps tails 2-4) ====
            for t in range(2):
                pst = psC.tile([128, 128], BF16, tag="Cb")
                nc.tensor.transpose(pst[:], Wm_sh[t][:], ident_bf[:])
                evac(WmT_sh[:, t * 128:(t + 1) * 128], pst[:], which="a")
                pst2 = psC.tile([128, 128], BF16, tag="Cb")
                nc.tensor.transpose(pst2[:], inter_sh[t][:], ident_bf[:])
                evac(intT_sh[:, t * 128:(t + 1) * 128], pst2[:], which="a")

            ag2_in = dramp.tile([2, 128, BSH], BF16)
            ag2_out = dramp.tile([NCORES, 2, 128, BSH], BF16, addr_space="Shared")
            nc.sync.dma_start(out=ag2_in[0], in_=WmT_sh[:])
            nc.sync.dma_start(out=ag2_in[1], in_=intT_sh[:])
            nc.gpsimd.collective_compute(
                "AllGather", ALU.bypass,
                replica_groups=[list(range(NCORES))],
                ins=[ag2_in.opt()], outs=[ag2_out.opt()])

            tail_group(2)
            tail_group(3)
            tail_group(4)
            psC.release()

            nc.sync.dma_start(
                out=WmT_all[:], in_=ag2_out[:, 0].rearrange("r d b -> d r b"))
            nc.sync.dma_start(
                out=intT_all[:], in_=ag2_out[:, 1].rearrange("r d b -> d r b"))

            # BN1 on gathered WmT (full B)
            bn1g_s = load_const(bn1g, [128, 1], "c13")
            bn1b_s = load_const(bn1b, [128, 1], "c14")
            st1 = smallp.tile([128, 4, 6], F32, tag="st1")
            for i in range(4):
                nc.vector.bn_stats(st1[:, i, :], WmT_all[:, i * 512:(i + 1) * 512])
            bn1_scale, bn1_shift = bn_finalize(st1[:], bn1g_s, bn1b_s)
            nc.vector.tensor_scalar(WmT_nb[:], WmT_all[:], bn1_scale[:, 0:1],
                                    bn1_shift[:, 0:1], op0=ALU.mult, op1=ALU.add)

            # =========== Phase 4: scores (quad ring psA/psA/psD = depth 3) ======
            psD = tc.alloc_tile_pool(name="psD", bufs=1, space="PSUM")
            NLAST = NSH - 4 * 512               # 452 valid cols in group 4
            quad_i = [0]

            def quad_tile():
                quad_i[0] += 1
                pool = psD if quad_i[0] % 3 == 0 else psA
                return pool.tile([128, 2, 512], F32, tag="D" if pool is psD else "A",
                                 name=f"psq{quad_i[0]}")

            for bt in range(NB_FULL):
                r0 = bt * 128
                for lhsT, outd, rhs in ((WmT_nb, tucker, tsT_s),
                                        (intT_all, poss, tamT_s)):
                    for half in range(2):
                        psq = quad_tile()
                        for gg in range(2):
                            g = half * 2 + gg
                            mm(psq[:, gg, :], lhsT[:, r0:r0 + 128],
                               rhs[:, g * 512:(g + 1) * 512])
                        stq = stagep.tile([128, 1024], BF16, tag="stq")
                        evac(stq[:], psq[:].rearrange("p g n -> p (g n)"))
                        nc.sync.dma_start(
                            out=outd[r0:r0 + 128, half * 1024:(half + 1) * 1024],
                            in_=stq[:])
                    ps4 = psB.tile([128, 512], F32, tag="B")
                    mm(ps4[:], lhsT[:, r0:r0 + 128], rhs[:, 4 * 512:5 * 512])
                    st4 = stagep.tile([128, 512], BF16, tag="st4")
                    evac(st4[:], ps4[:])
                    nc.sync.dma_start(out=outd[r0:r0 + 128, 2048:NSH],
                                      in_=st4[:, 0:NLAST])
            psD.release()

    nc.finalize()
    return nc


# ---------------------------------------------------------------------------
# host side
# ---------------------------------------------------------------------------

BF = np.dtype(ml_dtypes.bfloat16)


def _to_np(x, dt=np.float32):
    return np.ascontiguousarray(np.asarray(x), dtype=dt)


def prepare_in_maps(inputs):
    head = _to_np(inputs["head_vector"])        # [B, E]
    rel = _to_np(inputs["relation_vector"])     # [B, E]
    ridx = np.ascontiguousarray(np.asarray(inputs["relation_index"]).astype(np.int32))
    tailv = _to_np(inputs["tail_vector"])       # [N, E]
    codebook = _to_np(inputs["codebook"])       # [R2, C, C]
    core = _to_np(inputs["core"])               # [C, C, C]

    cbT_host = np.ascontiguousarray(
        codebook.transpose(0, 2, 1).reshape(2 * R2, C * C // 2)).astype(BF)
    core_dc_host = np.ascontiguousarray(
        core.transpose(0, 2, 1).reshape(C, C * C)).astype(BF)

    headT_full = np.ascontiguousarray(head.T).astype(BF)   # [E, B]
    relT_full = np.ascontiguousarray(rel.T).astype(BF)     # [E, B]
    tailT_full = np.ascontiguousarray(tailv.T).astype(BF)  # [E, N]

    def chunked_bias(k, nk):
        return np.ascontiguousarray(_to_np(inputs[k]).reshape(nk, 128).T)

    col = lambda k: _to_np(inputs[k]).reshape(128, 1)
    wcast = lambda k: _to_np(inputs[k]).astype(BF)
    weights_common = {
        "hsw1": wcast("hsw1"), "hsb1": chunked_bias("hsb1", 4),
        "hsw2": wcast("hsw2"), "hsb2": col("hsb2"),
        "rsw1": wcast("rsw1"), "rsb1": chunked_bias("rsb1", 4),
        "rsw2": wcast("rsw2"), "rsb2": col("rsb2"),
        "tsw1": wcast("tsw1"), "tsb1": chunked_bias("tsb1", 4),
        "tsw2": wcast("tsw2"), "tsb2": col("tsb2"),
        "taw1": wcast("taw1"), "tab1": chunked_bias("tab1", 4),
        "taw2": wcast("taw2"),
        "tab2": _to_np(inputs["tab2"]).reshape(1, C).astype(BF),
        "hrw1": wcast("hrw1"), "hrb1": chunked_bias("hrb1", 2),
        "hrw2": wcast("hrw2"), "hrb2": chunked_bias("hrb2", 2),
        "hrw3": wcast("hrw3"), "hrb3": col("hrb3"),
        "bn0g": col("bn0_g"), "bn0b": col("bn0_b"),
        "bn1g": col("bn1_g"), "bn1b": col("bn1_b"),
        "cbT": cbT_host, "core_dc": core_dc_host,
    }

    in_maps = []
    for k in range(NCORES):
        b0 = k * BSH
        n0 = k * NSH
        # rotate headT so THIS core's 256 b-columns come first; BN0 stats are
        # order-invariant and slices [0:256] are "my" shard on every core.
        headT_k = np.ascontiguousarray(np.roll(headT_full, -b0, axis=1))
        tailT_k = np.zeros((E, NPAD), BF)
        tailT_k[:, :NSH] = tailT_full[:, n0:n0 + NSH]
        m = dict(weights_common)
        m["headT"] = headT_k
        m["relT"] = np.ascontiguousarray(relT_full[:, b0:b0 + BSH])
        m["tailT"] = tailT_k
        ri = ridx[b0:b0 + BSH]
        m["relidx"] = np.ascontiguousarray(
            np.stack([2 * ri, 2 * ri + 1], axis=1))
        in_maps.append(m)
    return in_maps


def assemble_outputs(results):
    tuckers, posses = [], []
    for k in range(NCORES):
        r = results[k]
        tuckers.append(np.asarray(r["tucker"]).astype(np.float32))
        posses.append(np.asarray(r["poss"]).astype(np.float32))
    tucker_full = np.concatenate(tuckers, axis=1)
    poss_full = np.concatenate(posses, axis=1)
    return tucker_full, poss_full


def kernel(**inputs):
    if "prog" not in _PROG_CACHE:
        _PROG_CACHE["prog"] = build_program()
    nc = _PROG_CACHE["prog"]
    in_maps = prepare_in_maps(inputs)
    res = run_bass_kernel_spmd(nc, in_maps, list(range(NCORES)))
    return assemble_outputs(res.results)
